# revision 56
# baseline (speedup 1.0000x reference)
"""AttentionBlock (GroupNorm + 1x1-conv QKV + MHSA + proj + residual) on 8
Trainium2 NeuronCores via Bass/Tile.

Sharding: 8 cores = 4 samples x 2 query-halves. The host reorders each
sample's spatial columns so the core's query half occupies columns 0:2048;
keys/values cover all 4096 columns (attention is permutation-invariant over
keys). Each core computes GroupNorm statistics + full K/V, Q for its 2048
queries, attention, projection + residual for its half. No collectives.

Numerics: all attention-path matmuls run in fp8e4 with the DoubleRow perf
mode (0.5 cycles/row, two stacked contraction slots per step):
  - QKV/proj contract 256 channels as [128, 2, *]
  - scores per head contract dh=64 at partition base 64*(h%2) in head-pair
    tiles (k slot 1 is zeroed, q is read through a 0-stride broadcast AP,
    so the DoubleRow second slot contributes nothing)
  - AV contracts 256 keys (two 128-key blocks) per step
Softmax exp drains the score PSUM through the only two engines with a PSUM
port (GPSIMD has none on TRN2): ACT runs the real Exp into fp8e4, DVE runs
a Schraudolph bit-trick exp (u8 = s*c1 + c2 truncated, bitcast as fp8e4),
17/15 interleaved.  GPSIMD carries the SBUF-side work (x/w fp8 converts).
The fp8 ones-column in vT holds 1/32 so the softmax denominator stays in
fp8 range; the 32x is folded back in the output projection epilogue, and
the residual (+32x, exact f32 identity matmul) is accumulated into the
projection PSUM by the PE.  GroupNorm is folded into the QKV weights
(W' = W * a[c], bias = W^T b) so normalized activations never materialize;
the k bias is dropped (softmax is invariant to per-query constants), and
the v bias is folded through W_proj into the projection bias
(out = W_p(AV + b_v 1^T) + b_p = W_p AV + (W_p b_v + b_p)), removing all
32 ones-row accumulates from the V path.  The
per-query reciprocal row is broadcast across 64 partitions by an SP-queue
DMA whose latency is hidden by deferring the normalize-multiply into the
next head's window.  The attention path's absolute accuracy is relaxed
(fp8 + approx exp), which is safe here: the residual dominates the output
norm by ~60x.

All PSUM traffic is organized as [128, 512] single-bank tiles with a
6-deep score rotation (6 score banks + 2 AV banks = the full 16KB):
on hardware the PSUM write-after-write semaphore round-trip costs ~450ns
per reuse (measured; CoreSim models ~100ns), so shallow rotations gate
the PE far below its streaming rate.  CoreSim cost model: 218us/core;
measured on TRN2 (marginal per-iteration of a multi-pass NEFF, tunnel
dispatch overhead cancelled): ~460-505us/core.

The residual HW-vs-sim factor (~2.2x) appears only when score+exp and
AV+normalize run concurrently (HW A/B: scores+exp alone 193us, AV side
alone 119us, both 500us).  Falsified by direct HW experiment: pt/rb pool
depth, AV issue deferral (2/4/8/16), score-PSUM grain (3x1024 vs 6x512),
5+3 PSUM split, exp engine assignment (all-ACT/all-DVE/blocked), and
PE-broadcast vs DMA-broadcast softmax normalization.  Best measured
config is the one encoded in the module defaults below.
"""

import numpy as np

import concourse.bass as bass
import concourse.tile as tile
from concourse import mybir
from concourse.bass_utils import run_bass_kernel_spmd
from concourse.masks import make_identity
from concourse.tile import ScopedClock

# ---------------------------------------------------------------- constants
B, C, HGT, WID = 4, 256, 64, 64
N = HGT * WID            # 4096 spatial positions
NQ = N // 2              # query half per core
HEADS = 4
DH = C // HEADS          # 64
EPS = 1e-5
ATT_SCALE = (C * HEADS) ** (-0.5)   # 1/32
NCORES = 8

F32 = mybir.dt.float32
F32R = mybir.dt.float32r
BF16 = mybir.dt.bfloat16
FP8 = mybir.dt.float8e4
U8 = mybir.dt.uint8
DR = mybir.MatmulPerfMode.DoubleRow

# Schraudolph exp in fp8e4 bit-space: bits(exp(s*ATT_SCALE)) ~= s*EC1 + EC2
LOG2E = 1.4426950408889634
EC1 = ATT_SCALE * LOG2E * 8.0
EC2 = 56.06
PDEN = 1.0 / 32.0        # ones-column value; folded back in the epilogue

# per-(head, query-block) engine schedule for the 32 exp blocks of a
# window.  GPSIMD has no PSUM port on TRN2, so only ACT (A) and DVE (D)
# can read the score PSUM; 17A/15D balances their HW rates.
EXP_PAT = "AD" * 15 + "AA"

# probe knobs (timing experiments only): "noav" skips AV/normalize/proj,
# "noscores" feeds AV from 4 pre-memset pt tiles instead of scores+exp.
PROBE = ""
PTP_BUFS = 4
RBP_BUFS = 3
AV_DEFER = 2     # how many sc tiles ahead of each av step (16 = full window)
ST_BUFS = 6      # score PSUM banks; ST_BUFS + AV_BUFS <= 8 (16KB PSUM)
AV_BUFS = 2      # AV PSUM banks
NORM_MODE = "dma"  # "pe": broadcast 1/den via PE outer product; "dma":
                   # SP-queue broadcast DMA (measured fastest on HW)
SC_DR = True     # scores in DoubleRow (0.5 cyc/row) vs plain fp8 (1 cyc)
AV_REPEAT = 1    # accumulate each AV step this many times: numerator and
                 # denominator scale together, so the softmax output is
                 # unchanged -- pure PE keep-warm work (p-state probe)

# ------------------------------------------------- walrus multi-wait patch
# The external neuronxcc walrus rejects >2 sync waits on a CTRL (Drain)
# instruction; split the Tile exit-clock waits across nofuse sync NOPs.
_MAXW = 1


def _split_drain_and_barrier(self, tick_clock, wait_clock):
    nc = self.nc
    probe = nc.sync.nop(nofuse=True, hint="drain_wait_probe")
    wait_clock.add_sem_waits(probe.ins, ScopedClock({None: tick_clock.global_clock}))
    si = probe.ins.sync_info
    waits = list(si.on_wait) if si is not None else []
    probe.ins.sync_info = mybir.SyncInfo(on_wait=waits[:_MAXW], on_update=[])
    rest = waits[_MAXW:]
    for i in range(0, len(rest), _MAXW):
        nop = nc.sync.nop(nofuse=True, hint=f"drain_wait_{i}")
        nop.ins.sync_info = mybir.SyncInfo(on_wait=rest[i:i + _MAXW], on_update=[])
    nc.sync.drain()
    nc.all_engine_barrier()
    assert self.sems is not None
    popped = nc._tile_sem_poison_stack.pop()
    assert popped is self._sem_poison
    nc.clear_and_free_semaphores(list(self.sems.allocated().values()))
    nc.all_engine_barrier()


def _apply_drain_patch():
    tile.TileContext._drain_and_barrier = _split_drain_and_barrier


def _split_excess_waits(nc):
    """External walrus accepts only one sync wait per instruction; hoist
    excess waits onto same-engine nofuse NOPs inserted just before."""
    k = 0
    for bb in nc.m.functions[0].blocks:
        insts = bb.instructions
        i = 0
        while i < len(insts):
            inst = insts[i]
            si = inst.sync_info
            if si is not None and len(si.on_wait) > 1:
                waits = list(si.on_wait)
                inst.sync_info = mybir.SyncInfo(on_wait=waits[-1:],
                                                on_update=list(si.on_update))
                nops = []
                for w in waits[:-1]:
                    nop = mybir.InstNoOp(
                        name=f"I-wsplit{k}",
                        sync_info=mybir.SyncInfo(on_wait=[w], on_update=[]),
                        bass_nofuse=True,
                        engine=inst.engine,
                    )
                    k += 1
                    nops.append(nop)
                insts[i:i] = nops
                bb.instructions = insts
                i += len(nops)
            i += 1
    return k


# ------------------------------------------------------------- the program
def _body(tc, xbf, xf8, wqkv, wproj, gam, bet, bpr, inda, indb, zr8, out,
          mode="full"):
    nc = tc.nc
    AF = mybir.ActivationFunctionType
    OP = mybir.AluOpType
    do_front = mode in ("full", "noattn")
    do_attn = mode in ("full", "attnonly")

    persist_cm = tc.tile_pool(name="persist", bufs=1)
    work_cm = tc.tile_pool(name="work", bufs=2)
    persist = persist_cm.__enter__()
    work = work_cm.__enter__()

    # ---------------- load inputs (x split across 3 DMA queues).  x ships
    # from the host pre-converted: bf16 for GN stats + residual, fp8 for
    # the QKV matmuls (no on-device converts, half the f32 DMA bytes).
    x_t = persist.tile([128, 2, N], BF16)
    xbf_r = xbf.rearrange("(o p) n -> p o n", p=128)
    # chunk 0 is split in two so bn_stats can start one DMA-latency earlier
    x_chunks = [(0, 256), (256, 256)] + [(512 * c, 512) for c in range(1, 8)]
    chunk_eng = [nc.sync, nc.sync, nc.scalar, nc.sync, nc.scalar,
                 nc.sync, nc.scalar, nc.sync, nc.scalar]
    w_t = persist.tile([128, 2, 3 * C], F32)
    wp_t = persist.tile([128, 2, C], F32)
    gam_t = persist.tile([128, 2], F32)
    bet_t = persist.tile([128, 2], F32)
    bpr_t = persist.tile([128, 2], F32)
    x8 = persist.tile([128, 2, N], FP8)
    w8 = persist.tile([128, 2, 3 * C], FP8)
    wp8 = persist.tile([128, 2, C], FP8)
    if do_front:
        for (off, sz), eng in zip(x_chunks, chunk_eng):
            eng.dma_start(out=x_t[:, :, off:off + sz],
                          in_=xbf_r[:, :, off:off + sz])
        nc.scalar.dma_start(out=x8,
                            in_=xf8.rearrange("(o p) n -> p o n", p=128))
        nc.sync.dma_start(out=w_t,
                          in_=wqkv.rearrange("(o p) m -> p o m", p=128))
        nc.sync.dma_start(out=wp_t,
                          in_=wproj.rearrange("(o p) m -> p o m", p=128))
        nc.sync.dma_start(out=gam_t, in_=gam.rearrange("(o p) -> p o", p=128))
        nc.sync.dma_start(out=bet_t, in_=bet.rearrange("(o p) -> p o", p=128))
        nc.sync.dma_start(out=bpr_t, in_=bpr.rearrange("(o p) -> p o", p=128))
    out_r = out.rearrange("(o p) n -> p o n", p=128)

    # ---------------- phase 1: GroupNorm stats -> per-channel affine (a, b)
    qb = persist.tile([128, 6], F32)
    with tc.tile_pool(name="ps_small", bufs=1, space="PSUM") as ps_small:
        ident = persist.tile([128, 128], F32)
        make_identity(nc, ident)
        # bf16: 32.0 and the 0/32 entries are exact, and the residual
        # matmul runs at 1 cyc/row instead of f32's 4
        ident32 = persist.tile([128, 128], BF16)
        nc.vector.tensor_scalar_mul(out=ident32, in0=ident, scalar1=32.0)
        ones1b = persist.tile([1, 128], BF16)
        nc.vector.memset(ones1b, 1.0)
        if do_front:
            _gn_fold(tc, persist, work, ps_small, x_t, x_chunks, w_t, wp_t,
                     gam_t, bet_t, bpr_t, inda, indb, qb, x8, w8, wp8)
    _qkv_attn(tc, persist, work, x_t, x8, w8, wp8, qb, bpr_t, out_r, zr8,
              ident, ident32, ones1b, do_front, do_attn)

    work_cm.__exit__(None, None, None)
    persist_cm.__exit__(None, None, None)


def _gn_fold(tc, persist, work, ps_small, x_t, x_chunks, w_t, wp_t,
             gam_t, bet_t, bpr_t, inda, indb, qb, x8, w8, wp8):
    nc = tc.nc
    AF = mybir.ActivationFunctionType
    OP = mybir.AluOpType
    if True:
        # pre-warm the ACT sqrt table while DMAs run so the GN-path Sqrt
        # doesn't pay the 1.3us table load
        eps_t = persist.tile([8, 1], F32)
        nc.vector.memset(eps_t, EPS)
        warm = work.tile([8, 1], F32, tag="warm")
        nc.scalar.activation(out=warm, in_=eps_t, func=AF.Sqrt)

        # bn_stats on DVE for both channel halves (x8 ships pre-converted)
        stats6 = work.tile([128, 2, 9, 6], F32, tag="stats6")
        mv = work.tile([128, 2, 2], F32, tag="mv")
        for ch in range(2):
            for s, (off, sz) in enumerate(x_chunks):
                nc.vector.bn_stats(out=stats6[:, ch, s],
                                   in_=x_t[:, ch, off:off + sz])
            nc.vector.bn_aggr(out=mv[:, ch], in_=stats6[:, ch])
        nc.gpsimd.tensor_copy(out=wp8, in_=wp_t)
        # per-channel (mean, E[x^2])
        st2 = work.tile([128, 2, 2], F32, tag="st2")
        msq = work.tile([128, 1], F32, tag="msq")
        for ch in range(2):
            nc.vector.tensor_copy(out=st2[:, ch, 0:1], in_=mv[:, ch, 0:1])
            nc.vector.tensor_mul(out=msq, in0=mv[:, ch, 0:1], in1=mv[:, ch, 0:1])
            nc.vector.tensor_add(out=st2[:, ch, 1:2], in0=mv[:, ch, 1:2], in1=msq)

        # group reduce across channels: indicator matmul, values 1/32
        indA = persist.tile([128, 2, 8], F32)
        nc.sync.dma_start(out=indA, in_=inda.rearrange("(o p) g -> p o g", p=128))
        gps = ps_small.tile([128, 8], F32, tag="gps")
        for ch in range(2):
            nc.tensor.matmul(gps[0:8, 0:2], lhsT=indA[:, ch], rhs=st2[:, ch],
                             start=(ch == 0), stop=(ch == 1))
        # group var -> rstd;  gw cols: 0 mean, 1 rstd, 2 mean-work, 3 var-work
        gw = persist.tile([8, 4], F32)
        nc.vector.tensor_copy(out=gw[:, 2:4], in_=gps[0:8, 0:2])
        nc.vector.tensor_copy(out=gw[:, 0:1], in_=gw[:, 2:3])
        gmsq = work.tile([8, 1], F32, tag="gmsq")
        nc.vector.tensor_mul(out=gmsq, in0=gw[:, 2:3], in1=gw[:, 2:3])
        nc.vector.tensor_tensor(out=gw[:, 3:4], in0=gw[:, 3:4], in1=gmsq,
                                op=OP.subtract)
        nc.scalar.activation(out=gw[:, 3:4], in_=gw[:, 3:4], func=AF.Sqrt,
                             bias=eps_t)
        nc.vector.reciprocal(out=gw[:, 1:2], in_=gw[:, 3:4])

        # broadcast group (mean, rstd) back to channels
        indB = persist.tile([8, 2, 128], F32)
        nc.sync.dma_start(out=indB, in_=indb.rearrange("g (o p) -> g o p", p=128))
        chst = persist.tile([128, 2, 2], F32)   # [p, ch, {mean, rstd}]
        for ch in range(2):
            cp = ps_small.tile([128, 2], F32, tag="chps")
            nc.tensor.matmul(cp, lhsT=indB[:, ch], rhs=gw[:, 0:2],
                             start=True, stop=True)
            nc.vector.tensor_copy(out=chst[:, ch], in_=cp)

        # a = rstd * gamma ; b = beta - mean * a
        ab = persist.tile([128, 2, 2], F32)     # [p, ch, {a, b}]
        abt = work.tile([128, 1], F32, tag="abt")
        for ch in range(2):
            nc.vector.tensor_mul(out=ab[:, ch, 0:1], in0=chst[:, ch, 1:2],
                                 in1=gam_t[:, ch:ch + 1])
            nc.vector.tensor_mul(out=abt, in0=chst[:, ch, 0:1],
                                 in1=ab[:, ch, 0:1])
            nc.vector.tensor_tensor(out=ab[:, ch, 1:2], in0=bet_t[:, ch:ch + 1],
                                    in1=abt, op=OP.subtract)

        # ---------------- phase 2: fold GN into weights
        # qkv_bias[o] = sum_c W[o, c] * b[c]   (original W).  The k bias
        # (ob 2, 3) is unused: softmax is invariant to per-query constants.
        qbp = ps_small.tile([128, 8], F32, tag="qbp")
        for ob in (0, 1, 4, 5):
            for ch in range(2):
                nc.tensor.matmul(qbp[:, ob:ob + 1],
                                 lhsT=w_t[:, ch, 128 * ob:128 * (ob + 1)],
                                 rhs=ab[:, ch, 1:2],
                                 start=(ch == 0), stop=(ch == 1))
        nc.vector.tensor_copy(out=qb[:, 0:2], in_=qbp[:, 0:2])
        nc.vector.tensor_copy(out=qb[:, 4:6], in_=qbp[:, 4:6])
        # fold the v-bias through W_proj into the projection bias:
        # out = W_p(AV + b_v 1^T) + b_p = W_p AV + (W_p b_v + b_p).
        # qb cols 2:4 hold the combined projection bias; the V path then
        # needs no ones-row accumulate at all.
        wpp = ps_small.tile([128, 2], F32, tag="wpp")
        for ob in range(2):
            for ch in range(2):
                nc.tensor.matmul(wpp[:, ob:ob + 1],
                                 lhsT=wp_t[:, ch, 128 * ob:128 * (ob + 1)],
                                 rhs=qb[:, 4 + ch:5 + ch],
                                 start=(ch == 0), stop=(ch == 1))
        for ob in range(2):
            nc.vector.tensor_add(out=qb[:, 2 + ob:3 + ob],
                                 in0=wpp[:, ob:ob + 1],
                                 in1=bpr_t[:, ob:ob + 1])
        # W'[c, o] = W[c, o] * a[c], fused with the fp8 conversion
        # (q section on DVE so Q matmuls start first)
        for ch in range(2):
            nc.vector.tensor_scalar_mul(out=w8[:, ch, 0:256],
                                        in0=w_t[:, ch, 0:256],
                                        scalar1=ab[:, ch, 0:1])
            nc.gpsimd.tensor_scalar_mul(out=w8[:, ch, 256:768],
                                        in0=w_t[:, ch, 256:768],
                                        scalar1=ab[:, ch, 0:1])


def _qkv_attn(tc, persist, work, x_t, x8, w8, wp8, qb, bpr_t, out_r, zr8,
              ident, ident32, ones1b, do_front, do_attn):
    nc = tc.nc
    AF = mybir.ActivationFunctionType
    OP = mybir.AluOpType

    # ---------------- phase 3: QKV + attention, all fp8 DoubleRow
    # q/k head-pair tiles: partition = 64*(h%2) + d, tile index = h//2.
    # k slot 1 is zero so the scores DoubleRow second slot is inert.
    q_pair = [persist.tile([128, NQ], FP8, name=f"q_pair{i}")
              for i in range(2)]
    # k layout [p, head-pair, slot, n]; slot 1 is zero (DoubleRow inert slot)
    k_all = persist.tile([128, 2, 2, N], FP8)
    for pr in range(2):
        nc.sync.dma_start(out=k_all[:, pr, 1].bitcast(U8), in_=zr8[:, :])
    # vT layout [pos, key-block, head, 128]: DoubleRow ldweights requires the
    # slot-pair stride to be 128-aligned, so each head's 65 columns (64 dims
    # + the 1/32 denominator column) sit in their own 128-wide slot.
    vt_sb = persist.tile([128, 32, 4, 128], FP8)
    nc.vector.memset(vt_sb[:, :, :, 64:65], PDEN)
    ones64b = persist.tile([128, 64], BF16)
    nc.vector.memset(ones64b, 1.0)
    if not do_front:
        # attnonly probe: initialize everything the attention windows read
        # (idle engines only: SP-queue DMAs + Pool memsets)
        for i in range(2):
            nc.sync.dma_start(out=q_pair[i].bitcast(U8), in_=zr8[:, 0:NQ])
        for pr in range(2):
            nc.sync.dma_start(out=k_all[:, pr, 0].bitcast(U8), in_=zr8[:, :])
        nc.gpsimd.memset(vt_sb[:, :, :, 0:64], 0.015625)
        nc.gpsimd.memset(x_t, 0.5)
        nc.gpsimd.memset(wp8, 0.015625)
        nc.gpsimd.memset(bpr_t, 0.0)
        nc.gpsimd.memset(qb, 0.0)

    def q_bcast(h, ib):
        base = q_pair[h // 2][64 * (h % 2):64 * (h % 2) + 64,
                              512 * ib:512 * (ib + 1)]
        return bass.AP(tensor=base.tensor, offset=base.offset,
                       ap=[base.ap[0], [0, 2], base.ap[1]])

    with (
        tc.tile_pool(name="ps_st", bufs=ST_BUFS, space="PSUM") as ps_st,
        tc.tile_pool(name="ps_av", bufs=AV_BUFS, space="PSUM") as ps_av,
        tc.tile_pool(name="ptp", bufs=PTP_BUFS) as ptp,
        tc.tile_pool(name="atp", bufs=2) as atp,
        tc.tile_pool(name="rbp", bufs=RBP_BUFS) as rbp,
    ):
        # (v-bias is folded through W_proj into qb[:, 2:4] in _gn_fold,
        # so the V path needs no bias accumulate on the PE at all)

        # --- QKV projections.  All PSUM->SBUF drains alternate ACT/DVE
        # (GPSIMD has no PSUM port).
        cp_i = [0]

        def drain(dst, src, bias_col=None):
            a_turn = cp_i[0] % 2 == 0
            cp_i[0] += 1
            if a_turn:
                nc.scalar.activation(out=dst, in_=src, func=AF.Identity,
                                     bias=0.0 if bias_col is None else bias_col)
            elif bias_col is None:
                nc.vector.tensor_copy(out=dst, in_=src)
            else:
                nc.vector.tensor_scalar_add(out=dst, in0=src, scalar1=bias_col)

        def q_block(nb):
            for pr in range(2):
                st = ps_st.tile([128, 512], F32, tag="st")
                nc.tensor.matmul(st,
                                 lhsT=w8[:, :, 128 * pr:128 * (pr + 1)],
                                 rhs=x8[:, :, 512 * nb:512 * (nb + 1)],
                                 start=True, stop=True, perf_mode=DR)
                drain(q_pair[pr][:, 512 * nb:512 * (nb + 1)],
                      st, qb[:, pr:pr + 1])

        def k_block(nb):
            # k bias is dropped: softmax is invariant to per-query constants
            for pr in range(2):
                st = ps_st.tile([128, 512], F32, tag="st")
                nc.tensor.matmul(st,
                                 lhsT=w8[:, :, 256 + 128 * pr:256 + 128 * (pr + 1)],
                                 rhs=x8[:, :, 512 * nb:512 * (nb + 1)],
                                 start=True, stop=True, perf_mode=DR)
                drain(k_all[:, pr, 0, 512 * nb:512 * (nb + 1)], st)

        def v_group(g):
            # two 128-position blocks (256 v-channels each) per 512-col tile
            st = ps_st.tile([128, 512], F32, tag="st")
            for i in range(2):
                b = 2 * g + i
                nc.tensor.matmul(st[:, 256 * i:256 * (i + 1)],
                                 lhsT=x8[:, :, 128 * b:128 * (b + 1)],
                                 rhs=w8[:, :, 512:768],
                                 start=True, stop=True, perf_mode=DR)
            drain(vt_sb[:, 2 * g:2 * g + 2, :, 0:64],
                  st.rearrange("p (j h d) -> p j h d", j=2, d=64))

        if do_front:
            for nb in range(4):
                q_block(nb)
                k_block(nb)
                v_group(2 * nb)
                v_group(2 * nb + 1)
            for nb in range(4, 8):
                k_block(nb)
                v_group(2 * nb)
                v_group(2 * nb + 1)

        # --- attention
        def make_proj(ib, at2):
            def proj():
                sts = []
                for ob in range(2):
                    stx = ps_st.tile([128, 512], F32, tag="st")
                    sts.append(stx)
                    nc.tensor.matmul(stx,
                                     lhsT=wp8[:, :, 128 * ob:128 * (ob + 1)],
                                     rhs=at2, start=True, stop=False,
                                     perf_mode=DR)
                    # residual: st += 32 * x  (f32 identity matmul)
                    nc.tensor.matmul(
                        stx, lhsT=ident32,
                        rhs=x_t[:, ob, 512 * ib:512 * (ib + 1)],
                        start=False, stop=True)
                for ob in range(2):
                    ot = work.tile([128, 512], F32, tag="ot")
                    nc.scalar.activation(out=ot, in_=sts[ob],
                                         func=AF.Identity, scale=PDEN,
                                         bias=qb[:, 2 + ob:3 + ob])
                    nc.sync.dma_start(
                        out=out_r[:, ob, 512 * ib:512 * (ib + 1)], in_=ot)
            return proj

        if not do_attn:
            for ib in range(4):
                at2 = atp.tile([128, 2, 512], FP8, tag="at2")
                nc.gpsimd.memset(at2, 0.015625)
                make_proj(ib, at2)()
            return

        noav = "noav" in PROBE
        noscores = "noscores" in PROBE
        densepe = "densepe" in PROBE
        pt4 = None
        if noscores or densepe:
            pt4 = []
            for i in range(4):
                t_ = ptp.tile([128, 2, 512], FP8, tag="pt", name="pt")
                nc.gpsimd.memset(t_, 0.0078125)
                pt4.append(t_)

        pend_proj = None
        pend_norm = None
        for ib in range(4):
            at2 = atp.tile([128, 2, 512], FP8, tag="at2")
            for h in range(4):
                av = ps_av.tile([65, 512], F32, tag="av")
                pend = []

                def sc_pair(t, h=h, ib=ib):
                    if noscores:
                        return pt4[t % 4]
                    pt = ptp.tile([128, 2, 512], FP8, tag="pt", name="pt")
                    hh = h % 2
                    for u in range(2):
                        kb = 2 * t + u
                        st = ps_st.tile([128, 512], F32, tag="st")
                        if SC_DR:
                            nc.tensor.matmul(
                                st,
                                lhsT=k_all[64 * hh:64 * (hh + 1), h // 2, :,
                                           128 * kb:128 * (kb + 1)],
                                rhs=q_bcast(h, ib),
                                start=True, stop=True, perf_mode=DR)
                        else:
                            nc.tensor.matmul(
                                st,
                                lhsT=k_all[64 * hh:64 * (hh + 1), h // 2, 0,
                                           128 * kb:128 * (kb + 1)],
                                rhs=q_pair[h // 2][
                                    64 * hh:64 * (hh + 1),
                                    512 * ib:512 * (ib + 1)],
                                start=True, stop=True)
                        if EXP_PAT[kb] == "A":
                            nc.scalar.activation(out=pt[:, u], in_=st,
                                                 func=AF.Exp,
                                                 scale=ATT_SCALE)
                        else:
                            nc.vector.tensor_scalar(
                                out=pt[:, u].bitcast(U8), in0=st,
                                scalar1=EC1, scalar2=EC2,
                                op0=OP.mult, op1=OP.add)
                    return pt

                def av_step(t, pt, av=av, h=h):
                    if noav:
                        return
                    if densepe:
                        pt = pt4[t % 4]
                    for r in range(AV_REPEAT):
                        nc.tensor.matmul(
                            av, lhsT=vt_sb[:, 2 * t:2 * t + 2, h, 0:65],
                            rhs=pt,
                            start=(t == 0 and r == 0),
                            stop=(t == 15 and r == AV_REPEAT - 1),
                            perf_mode=DR)

                for t in range(16):
                    pend.append((t, sc_pair(t)))
                    if t == 4 and pend_norm is not None:
                        pend_norm()
                        pend_norm = None
                    if t == 13 and pend_proj is not None:
                        pend_proj()
                        pend_proj = None
                    if t >= AV_DEFER:
                        av_step(*pend.pop(0))
                while pend:
                    av_step(*pend.pop(0))
                if noav:
                    continue

                # normalize: at2[d, i] = av[d, i] * (32 / den[i]).  The
                # reciprocal + broadcast issue now; the multiply is deferred
                # into the next head's window to hide the broadcast latency.
                rec_b = work.tile([65, 512], BF16, tag="rec_b")
                with nc.allow_low_precision(reason="bf16 softmax denom"):
                    nc.vector.reciprocal(out=rec_b[64:65], in_=av[64:65])
                last = ib == 3 and h == 3
                if NORM_MODE == "pe" or last:
                    # broadcast on the PE (outer product into a spare av
                    # bank) and stage through SBUF for the multiply: no DMA
                    # and no 900ns DMA-completion semaphore in the window
                    # steady state.
                    rbq = ps_av.tile([65, 512], F32, tag="av", name="rbq")
                    nc.tensor.matmul(rbq[0:64], lhsT=ones64b[64:65, :],
                                     rhs=rec_b[64:65], start=True, stop=True)

                    def mult(av=av, rbq=rbq, h=h, at2=at2):
                        rbs = work.tile([64, 512], BF16, tag="rbs")
                        nc.vector.tensor_copy(out=rbs, in_=rbq[0:64])
                        nc.vector.tensor_mul(
                            out=at2[64 * (h % 2):64 * (h % 2) + 64, h // 2],
                            in0=av[0:64], in1=rbs)
                    if last:
                        mult()
                        break
                else:
                    rb = rbp.tile([64, 512], BF16, tag="rb")
                    rsrc = rec_b[64:65]
                    nc.sync.dma_start(out=rb, in_=bass.AP(
                        tensor=rsrc.tensor, offset=rsrc.offset,
                        ap=[list(rsrc.ap[0]), [0, 64]] + list(rsrc.ap[1:])))

                    def mult(av=av, rb=rb, h=h, at2=at2):
                        nc.vector.tensor_mul(
                            out=at2[64 * (h % 2):64 * (h % 2) + 64, h // 2],
                            in0=av[0:64], in1=rb)
                pend_norm = mult

            if not noav:
                pend_proj = make_proj(ib, at2)
        if pend_norm is not None:
            pend_norm()
            pend_norm = None
        if pend_proj is not None:
            pend_proj()


def build_program(split_waits=True, iters=1, mode="full"):
    _apply_drain_patch()
    nc = bass.Bass()
    xbf = nc.declare_dram_parameter("x_bf16", [C, N], BF16, isOutput=False)
    xf8 = nc.declare_dram_parameter("x_fp8", [C, N], FP8, isOutput=False)
    wqkv = nc.declare_dram_parameter("w_qkvT", [C, 3 * C], F32, isOutput=False)
    wproj = nc.declare_dram_parameter("w_projT", [C, C], F32, isOutput=False)
    gam = nc.declare_dram_parameter("gn_gamma", [C], F32, isOutput=False)
    bet = nc.declare_dram_parameter("gn_beta", [C], F32, isOutput=False)
    bpr = nc.declare_dram_parameter("b_proj", [C], F32, isOutput=False)
    inda = nc.declare_dram_parameter("indA", [C, 8], F32, isOutput=False)
    indb = nc.declare_dram_parameter("indB", [8, C], F32, isOutput=False)
    zr8 = nc.declare_dram_parameter("zeros8", [128, N], mybir.dt.uint8,
                                    isOutput=False)
    out = nc.declare_dram_parameter("out", [C, NQ], F32, isOutput=True)
    with tile.TileContext(nc) as tc:
        for _ in range(iters):
            _body(tc, xbf, xf8, wqkv, wproj, gam, bet, bpr, inda, indb, zr8,
                  out, mode=mode)
    if split_waits:
        _split_excess_waits(nc)
    return nc


def make_in_maps(x, gn_gamma, gn_beta, w_qkv, w_proj, b_proj):
    x = np.ascontiguousarray(x, dtype=np.float32)
    w_qkvT = np.ascontiguousarray(np.asarray(w_qkv, np.float32).T)
    w_projT = np.ascontiguousarray(np.asarray(w_proj, np.float32).T)
    gn_gamma = np.ascontiguousarray(gn_gamma, dtype=np.float32)
    gn_beta = np.ascontiguousarray(gn_beta, dtype=np.float32)
    b_proj = np.ascontiguousarray(b_proj, dtype=np.float32)
    ch_groups = np.arange(C) // 32
    indA = np.zeros((C, 8), np.float32)
    indA[np.arange(C), ch_groups] = 1.0 / 32.0
    indB = np.zeros((8, C), np.float32)
    indB[ch_groups, np.arange(C)] = 1.0
    in_maps = []
    bf16_np = mybir.dt.np(BF16)
    fp8_np = mybir.dt.np(FP8)
    for core in range(NCORES):
        s, half = core // 2, core % 2
        xfl = x[s].reshape(C, N)
        x_core = np.ascontiguousarray(np.concatenate(
            [xfl[:, half * NQ:(half + 1) * NQ],
             xfl[:, (1 - half) * NQ:(2 - half) * NQ]], axis=1))
        in_maps.append({
            "x_bf16": x_core.astype(bf16_np),
            "x_fp8": x_core.astype(fp8_np),
            "w_qkvT": w_qkvT,
            "w_projT": w_projT,
            "gn_gamma": gn_gamma,
            "gn_beta": gn_beta,
            "b_proj": b_proj,
            "indA": indA,
            "indB": indB,
            "zeros8": np.zeros((128, N), np.uint8),
        })
    return in_maps


def assemble_output(results):
    out = np.empty((B, C, N), np.float32)
    for core in range(NCORES):
        s, half = core // 2, core % 2
        out[s][:, half * NQ:(half + 1) * NQ] = results[core]["out"]
    return out.reshape(B, C, HGT, WID)


_PROGRAM_CACHE = {}


def kernel(x, gn_gamma, gn_beta, w_qkv, w_proj, b_proj):
    if "nc" not in _PROGRAM_CACHE:
        _PROGRAM_CACHE["nc"] = build_program()
    nc = _PROGRAM_CACHE["nc"]
    in_maps = make_in_maps(x, gn_gamma, gn_beta, w_qkv, w_proj, b_proj)
    res = run_bass_kernel_spmd(nc, in_maps, list(range(NCORES)))
    return assemble_output(res.results)



# revision 60
# speedup vs baseline: 1.0105x; 1.0105x over previous
"""AttentionBlock (GroupNorm + 1x1-conv QKV + MHSA + proj + residual) on 8
Trainium2 NeuronCores via Bass/Tile.

Sharding: 8 cores = 4 samples x 2 query-halves. The host reorders each
sample's spatial columns so the core's query half occupies columns 0:2048;
keys/values cover all 4096 columns (attention is permutation-invariant over
keys). Each core computes GroupNorm statistics + full K/V, Q for its 2048
queries, attention, projection + residual for its half. No collectives.

Numerics: all attention-path matmuls run in fp8e4 with the DoubleRow perf
mode (0.5 cycles/row, two stacked contraction slots per step):
  - QKV/proj contract 256 channels as [128, 2, *]
  - scores per head contract dh=64 at partition base 64*(h%2) in head-pair
    tiles (k slot 1 is zeroed, q is read through a 0-stride broadcast AP,
    so the DoubleRow second slot contributes nothing)
  - AV contracts 256 keys (two 128-key blocks) per step
Softmax exp drains the score PSUM through the only two engines with a PSUM
port (GPSIMD has none on TRN2): ACT runs the real Exp into fp8e4, DVE runs
a Schraudolph bit-trick exp (u8 = s*c1 + c2 truncated, bitcast as fp8e4),
17/15 interleaved.  GPSIMD carries the SBUF-side work (x/w fp8 converts).
The fp8 ones-column in vT holds 1/32 so the softmax denominator stays in
fp8 range; the 32x is folded back in the output projection epilogue, and
the residual (+32x, exact f32 identity matmul) is accumulated into the
projection PSUM by the PE.  GroupNorm is folded into the QKV weights
(W' = W * a[c], bias = W^T b) so normalized activations never materialize;
the k bias is dropped (softmax is invariant to per-query constants), and
the v bias is folded through W_proj into the projection bias
(out = W_p(AV + b_v 1^T) + b_p = W_p AV + (W_p b_v + b_p)), removing all
32 ones-row accumulates from the V path.  The
per-query reciprocal row is broadcast across 64 partitions by an SP-queue
DMA whose latency is hidden by deferring the normalize-multiply into the
next head's window.  The attention path's absolute accuracy is relaxed
(fp8 + approx exp), which is safe here: the residual dominates the output
norm by ~60x.

All PSUM traffic is organized as [128, 512] single-bank tiles with a
6-deep score rotation (6 score banks + 2 AV banks = the full 16KB):
on hardware the PSUM write-after-write semaphore round-trip costs ~450ns
per reuse (measured; CoreSim models ~100ns), so shallow rotations gate
the PE far below its streaming rate.  CoreSim cost model: 218us/core;
measured on TRN2 (marginal per-iteration of a multi-pass NEFF, tunnel
dispatch overhead cancelled): ~460-505us/core.

The residual HW-vs-sim factor (~2.2x) appears only when score+exp and
AV+normalize run concurrently (HW A/B: scores+exp alone 193us, AV side
alone 119us, both 500us).  Falsified by direct HW experiment: pt/rb pool
depth, AV issue deferral (2/4/8/16), score-PSUM grain (3x1024 vs 6x512),
5+3 PSUM split, exp engine assignment (all-ACT/all-DVE/blocked), and
PE-broadcast vs DMA-broadcast softmax normalization.  Best measured
config is the one encoded in the module defaults below.
"""

import numpy as np

import concourse.bass as bass
import concourse.tile as tile
from concourse import mybir
from concourse.bass_utils import run_bass_kernel_spmd
from concourse.masks import make_identity
from concourse.tile import ScopedClock

# ---------------------------------------------------------------- constants
B, C, HGT, WID = 4, 256, 64, 64
N = HGT * WID            # 4096 spatial positions
NQ = N // 2              # query half per core
HEADS = 4
DH = C // HEADS          # 64
EPS = 1e-5
ATT_SCALE = (C * HEADS) ** (-0.5)   # 1/32
NCORES = 8

F32 = mybir.dt.float32
F32R = mybir.dt.float32r
BF16 = mybir.dt.bfloat16
FP8 = mybir.dt.float8e4
U8 = mybir.dt.uint8
DR = mybir.MatmulPerfMode.DoubleRow

# Schraudolph exp in fp8e4 bit-space: bits(exp(s*ATT_SCALE)) ~= s*EC1 + EC2
LOG2E = 1.4426950408889634
EC1 = ATT_SCALE * LOG2E * 8.0
EC2 = 56.06
PDEN = 1.0 / 32.0        # ones-column value; folded back in the epilogue

# per-(head, query-block) engine schedule for the 32 exp blocks of a
# window.  GPSIMD has no PSUM port on TRN2, so only ACT (A) and DVE (D)
# can read the score PSUM; 17A/15D balances their HW rates.
EXP_PAT = "AD" * 15 + "AA"

# probe knobs (timing experiments only): "noav" skips AV/normalize/proj,
# "noscores" feeds AV from 4 pre-memset pt tiles instead of scores+exp.
PROBE = ""
PTP_BUFS = 6
RBP_BUFS = 3
AV_DEFER = 2     # how many sc tiles ahead of each av step (16 = full window)
AV_GROUP = 2     # issue av steps in back-to-back pairs: halves the PE's
                 # PSUM accumulate-bank reopens (measured ~45us on HW;
                 # groups of 4 are worse -- latency outweighs the saving)
ST_BUFS = 6      # score PSUM banks; ST_BUFS + AV_BUFS <= 8 (16KB PSUM)
AV_BUFS = 2      # AV PSUM banks
NORM_MODE = "dma"  # "pe": broadcast 1/den via PE outer product; "dma":
                   # SP-queue broadcast DMA (measured fastest on HW)
SC_DR = True     # scores in DoubleRow (0.5 cyc/row) vs plain fp8 (1 cyc)
AV_REPEAT = 1    # accumulate each AV step this many times: numerator and
                 # denominator scale together, so the softmax output is
                 # unchanged -- pure PE keep-warm work (p-state probe)

# ------------------------------------------------- walrus multi-wait patch
# The external neuronxcc walrus rejects >2 sync waits on a CTRL (Drain)
# instruction; split the Tile exit-clock waits across nofuse sync NOPs.
_MAXW = 1


def _split_drain_and_barrier(self, tick_clock, wait_clock):
    nc = self.nc
    probe = nc.sync.nop(nofuse=True, hint="drain_wait_probe")
    wait_clock.add_sem_waits(probe.ins, ScopedClock({None: tick_clock.global_clock}))
    si = probe.ins.sync_info
    waits = list(si.on_wait) if si is not None else []
    probe.ins.sync_info = mybir.SyncInfo(on_wait=waits[:_MAXW], on_update=[])
    rest = waits[_MAXW:]
    for i in range(0, len(rest), _MAXW):
        nop = nc.sync.nop(nofuse=True, hint=f"drain_wait_{i}")
        nop.ins.sync_info = mybir.SyncInfo(on_wait=rest[i:i + _MAXW], on_update=[])
    nc.sync.drain()
    nc.all_engine_barrier()
    assert self.sems is not None
    popped = nc._tile_sem_poison_stack.pop()
    assert popped is self._sem_poison
    nc.clear_and_free_semaphores(list(self.sems.allocated().values()))
    nc.all_engine_barrier()


def _apply_drain_patch():
    tile.TileContext._drain_and_barrier = _split_drain_and_barrier


def _split_excess_waits(nc):
    """External walrus accepts only one sync wait per instruction; hoist
    excess waits onto same-engine nofuse NOPs inserted just before."""
    k = 0
    for bb in nc.m.functions[0].blocks:
        insts = bb.instructions
        i = 0
        while i < len(insts):
            inst = insts[i]
            si = inst.sync_info
            if si is not None and len(si.on_wait) > 1:
                waits = list(si.on_wait)
                inst.sync_info = mybir.SyncInfo(on_wait=waits[-1:],
                                                on_update=list(si.on_update))
                nops = []
                for w in waits[:-1]:
                    nop = mybir.InstNoOp(
                        name=f"I-wsplit{k}",
                        sync_info=mybir.SyncInfo(on_wait=[w], on_update=[]),
                        bass_nofuse=True,
                        engine=inst.engine,
                    )
                    k += 1
                    nops.append(nop)
                insts[i:i] = nops
                bb.instructions = insts
                i += len(nops)
            i += 1
    return k


# ------------------------------------------------------------- the program
def _body(tc, xbf, xf8, wqkv, wproj, gam, bet, bpr, inda, indb, zr8, out,
          mode="full"):
    nc = tc.nc
    AF = mybir.ActivationFunctionType
    OP = mybir.AluOpType
    do_front = mode in ("full", "noattn")
    do_attn = mode in ("full", "attnonly")

    persist_cm = tc.tile_pool(name="persist", bufs=1)
    work_cm = tc.tile_pool(name="work", bufs=2)
    persist = persist_cm.__enter__()
    work = work_cm.__enter__()

    # ---------------- load inputs (x split across 3 DMA queues).  x ships
    # from the host pre-converted: bf16 for GN stats + residual, fp8 for
    # the QKV matmuls (no on-device converts, half the f32 DMA bytes).
    x_t = persist.tile([128, 2, N], BF16)
    xbf_r = xbf.rearrange("(o p) n -> p o n", p=128)
    # chunk 0 is split in two so bn_stats can start one DMA-latency earlier
    x_chunks = [(0, 256), (256, 256)] + [(512 * c, 512) for c in range(1, 8)]
    chunk_eng = [nc.sync, nc.sync, nc.scalar, nc.sync, nc.scalar,
                 nc.sync, nc.scalar, nc.sync, nc.scalar]
    w_t = persist.tile([128, 2, 3 * C], F32)
    wp_t = persist.tile([128, 2, C], F32)
    gam_t = persist.tile([128, 2], F32)
    bet_t = persist.tile([128, 2], F32)
    bpr_t = persist.tile([128, 2], F32)
    x8 = persist.tile([128, 2, N], FP8)
    w8 = persist.tile([128, 2, 3 * C], FP8)
    wp8 = persist.tile([128, 2, C], FP8)
    if do_front:
        for (off, sz), eng in zip(x_chunks, chunk_eng):
            eng.dma_start(out=x_t[:, :, off:off + sz],
                          in_=xbf_r[:, :, off:off + sz])
        nc.scalar.dma_start(out=x8,
                            in_=xf8.rearrange("(o p) n -> p o n", p=128))
        nc.sync.dma_start(out=w_t,
                          in_=wqkv.rearrange("(o p) m -> p o m", p=128))
        nc.sync.dma_start(out=wp_t,
                          in_=wproj.rearrange("(o p) m -> p o m", p=128))
        nc.sync.dma_start(out=gam_t, in_=gam.rearrange("(o p) -> p o", p=128))
        nc.sync.dma_start(out=bet_t, in_=bet.rearrange("(o p) -> p o", p=128))
        nc.sync.dma_start(out=bpr_t, in_=bpr.rearrange("(o p) -> p o", p=128))
    out_r = out.rearrange("(o p) n -> p o n", p=128)

    # ---------------- phase 1: GroupNorm stats -> per-channel affine (a, b)
    qb = persist.tile([128, 6], F32)
    with tc.tile_pool(name="ps_small", bufs=1, space="PSUM") as ps_small:
        ident = persist.tile([128, 128], F32)
        make_identity(nc, ident)
        # bf16: 32.0 and the 0/32 entries are exact, and the residual
        # matmul runs at 1 cyc/row instead of f32's 4
        ident32 = persist.tile([128, 128], BF16)
        nc.vector.tensor_scalar_mul(out=ident32, in0=ident, scalar1=32.0)
        ones1b = persist.tile([1, 128], BF16)
        nc.vector.memset(ones1b, 1.0)
        if do_front:
            _gn_fold(tc, persist, work, ps_small, x_t, x_chunks, w_t, wp_t,
                     gam_t, bet_t, bpr_t, inda, indb, qb, x8, w8, wp8)
    _qkv_attn(tc, persist, work, x_t, x8, w8, wp8, qb, bpr_t, out_r, zr8,
              ident, ident32, ones1b, do_front, do_attn)

    work_cm.__exit__(None, None, None)
    persist_cm.__exit__(None, None, None)


def _gn_fold(tc, persist, work, ps_small, x_t, x_chunks, w_t, wp_t,
             gam_t, bet_t, bpr_t, inda, indb, qb, x8, w8, wp8):
    nc = tc.nc
    AF = mybir.ActivationFunctionType
    OP = mybir.AluOpType
    if True:
        # pre-warm the ACT sqrt table while DMAs run so the GN-path Sqrt
        # doesn't pay the 1.3us table load
        eps_t = persist.tile([8, 1], F32)
        nc.vector.memset(eps_t, EPS)
        warm = work.tile([8, 1], F32, tag="warm")
        nc.scalar.activation(out=warm, in_=eps_t, func=AF.Sqrt)

        # bn_stats on DVE for both channel halves (x8 ships pre-converted)
        stats6 = work.tile([128, 2, 9, 6], F32, tag="stats6")
        mv = work.tile([128, 2, 2], F32, tag="mv")
        for ch in range(2):
            for s, (off, sz) in enumerate(x_chunks):
                nc.vector.bn_stats(out=stats6[:, ch, s],
                                   in_=x_t[:, ch, off:off + sz])
            nc.vector.bn_aggr(out=mv[:, ch], in_=stats6[:, ch])
        nc.gpsimd.tensor_copy(out=wp8, in_=wp_t)
        # per-channel (mean, E[x^2])
        st2 = work.tile([128, 2, 2], F32, tag="st2")
        msq = work.tile([128, 1], F32, tag="msq")
        for ch in range(2):
            nc.vector.tensor_copy(out=st2[:, ch, 0:1], in_=mv[:, ch, 0:1])
            nc.vector.tensor_mul(out=msq, in0=mv[:, ch, 0:1], in1=mv[:, ch, 0:1])
            nc.vector.tensor_add(out=st2[:, ch, 1:2], in0=mv[:, ch, 1:2], in1=msq)

        # group reduce across channels: indicator matmul, values 1/32
        indA = persist.tile([128, 2, 8], F32)
        nc.sync.dma_start(out=indA, in_=inda.rearrange("(o p) g -> p o g", p=128))
        gps = ps_small.tile([128, 8], F32, tag="gps")
        for ch in range(2):
            nc.tensor.matmul(gps[0:8, 0:2], lhsT=indA[:, ch], rhs=st2[:, ch],
                             start=(ch == 0), stop=(ch == 1))
        # group var -> rstd;  gw cols: 0 mean, 1 rstd, 2 mean-work, 3 var-work
        gw = persist.tile([8, 4], F32)
        nc.vector.tensor_copy(out=gw[:, 2:4], in_=gps[0:8, 0:2])
        nc.vector.tensor_copy(out=gw[:, 0:1], in_=gw[:, 2:3])
        gmsq = work.tile([8, 1], F32, tag="gmsq")
        nc.vector.tensor_mul(out=gmsq, in0=gw[:, 2:3], in1=gw[:, 2:3])
        nc.vector.tensor_tensor(out=gw[:, 3:4], in0=gw[:, 3:4], in1=gmsq,
                                op=OP.subtract)
        nc.scalar.activation(out=gw[:, 3:4], in_=gw[:, 3:4], func=AF.Sqrt,
                             bias=eps_t)
        nc.vector.reciprocal(out=gw[:, 1:2], in_=gw[:, 3:4])

        # broadcast group (mean, rstd) back to channels
        indB = persist.tile([8, 2, 128], F32)
        nc.sync.dma_start(out=indB, in_=indb.rearrange("g (o p) -> g o p", p=128))
        chst = persist.tile([128, 2, 2], F32)   # [p, ch, {mean, rstd}]
        for ch in range(2):
            cp = ps_small.tile([128, 2], F32, tag="chps")
            nc.tensor.matmul(cp, lhsT=indB[:, ch], rhs=gw[:, 0:2],
                             start=True, stop=True)
            nc.vector.tensor_copy(out=chst[:, ch], in_=cp)

        # a = rstd * gamma ; b = beta - mean * a
        ab = persist.tile([128, 2, 2], F32)     # [p, ch, {a, b}]
        abt = work.tile([128, 1], F32, tag="abt")
        for ch in range(2):
            nc.vector.tensor_mul(out=ab[:, ch, 0:1], in0=chst[:, ch, 1:2],
                                 in1=gam_t[:, ch:ch + 1])
            nc.vector.tensor_mul(out=abt, in0=chst[:, ch, 0:1],
                                 in1=ab[:, ch, 0:1])
            nc.vector.tensor_tensor(out=ab[:, ch, 1:2], in0=bet_t[:, ch:ch + 1],
                                    in1=abt, op=OP.subtract)

        # ---------------- phase 2: fold GN into weights
        # qkv_bias[o] = sum_c W[o, c] * b[c]   (original W).  The k bias
        # (ob 2, 3) is unused: softmax is invariant to per-query constants.
        qbp = ps_small.tile([128, 8], F32, tag="qbp")
        for ob in (0, 1, 4, 5):
            for ch in range(2):
                nc.tensor.matmul(qbp[:, ob:ob + 1],
                                 lhsT=w_t[:, ch, 128 * ob:128 * (ob + 1)],
                                 rhs=ab[:, ch, 1:2],
                                 start=(ch == 0), stop=(ch == 1))
        nc.vector.tensor_copy(out=qb[:, 0:2], in_=qbp[:, 0:2])
        nc.vector.tensor_copy(out=qb[:, 4:6], in_=qbp[:, 4:6])
        # fold the v-bias through W_proj into the projection bias:
        # out = W_p(AV + b_v 1^T) + b_p = W_p AV + (W_p b_v + b_p).
        # qb cols 2:4 hold the combined projection bias; the V path then
        # needs no ones-row accumulate at all.
        wpp = ps_small.tile([128, 2], F32, tag="wpp")
        for ob in range(2):
            for ch in range(2):
                nc.tensor.matmul(wpp[:, ob:ob + 1],
                                 lhsT=wp_t[:, ch, 128 * ob:128 * (ob + 1)],
                                 rhs=qb[:, 4 + ch:5 + ch],
                                 start=(ch == 0), stop=(ch == 1))
        for ob in range(2):
            nc.vector.tensor_add(out=qb[:, 2 + ob:3 + ob],
                                 in0=wpp[:, ob:ob + 1],
                                 in1=bpr_t[:, ob:ob + 1])
        # W'[c, o] = W[c, o] * a[c], fused with the fp8 conversion
        # (q section on DVE so Q matmuls start first)
        for ch in range(2):
            nc.vector.tensor_scalar_mul(out=w8[:, ch, 0:256],
                                        in0=w_t[:, ch, 0:256],
                                        scalar1=ab[:, ch, 0:1])
            nc.gpsimd.tensor_scalar_mul(out=w8[:, ch, 256:768],
                                        in0=w_t[:, ch, 256:768],
                                        scalar1=ab[:, ch, 0:1])


def _qkv_attn(tc, persist, work, x_t, x8, w8, wp8, qb, bpr_t, out_r, zr8,
              ident, ident32, ones1b, do_front, do_attn):
    nc = tc.nc
    AF = mybir.ActivationFunctionType
    OP = mybir.AluOpType

    # ---------------- phase 3: QKV + attention, all fp8 DoubleRow
    # q/k head-pair tiles: partition = 64*(h%2) + d, tile index = h//2.
    # k slot 1 is zero so the scores DoubleRow second slot is inert.
    q_pair = [persist.tile([128, NQ], FP8, name=f"q_pair{i}")
              for i in range(2)]
    # k layout [p, head-pair, slot, n]; slot 1 is zero (DoubleRow inert slot)
    k_all = persist.tile([128, 2, 2, N], FP8)
    for pr in range(2):
        nc.sync.dma_start(out=k_all[:, pr, 1].bitcast(U8), in_=zr8[:, :])
    # vT layout [pos, key-block, head, 128]: DoubleRow ldweights requires the
    # slot-pair stride to be 128-aligned, so each head's 65 columns (64 dims
    # + the 1/32 denominator column) sit in their own 128-wide slot.
    vt_sb = persist.tile([128, 32, 4, 128], FP8)
    nc.vector.memset(vt_sb[:, :, :, 64:65], PDEN)
    ones64b = persist.tile([128, 64], BF16)
    nc.vector.memset(ones64b, 1.0)
    if not do_front:
        # attnonly probe: initialize everything the attention windows read
        # (idle engines only: SP-queue DMAs + Pool memsets)
        for i in range(2):
            nc.sync.dma_start(out=q_pair[i].bitcast(U8), in_=zr8[:, 0:NQ])
        for pr in range(2):
            nc.sync.dma_start(out=k_all[:, pr, 0].bitcast(U8), in_=zr8[:, :])
        nc.gpsimd.memset(vt_sb[:, :, :, 0:64], 0.015625)
        nc.gpsimd.memset(x_t, 0.5)
        nc.gpsimd.memset(wp8, 0.015625)
        nc.gpsimd.memset(bpr_t, 0.0)
        nc.gpsimd.memset(qb, 0.0)

    def q_bcast(h, ib):
        base = q_pair[h // 2][64 * (h % 2):64 * (h % 2) + 64,
                              512 * ib:512 * (ib + 1)]
        return bass.AP(tensor=base.tensor, offset=base.offset,
                       ap=[base.ap[0], [0, 2], base.ap[1]])

    with (
        tc.tile_pool(name="ps_st", bufs=ST_BUFS, space="PSUM") as ps_st,
        tc.tile_pool(name="ps_av", bufs=AV_BUFS, space="PSUM") as ps_av,
        tc.tile_pool(name="ptp", bufs=PTP_BUFS) as ptp,
        tc.tile_pool(name="atp", bufs=2) as atp,
        tc.tile_pool(name="rbp", bufs=RBP_BUFS) as rbp,
    ):
        # (v-bias is folded through W_proj into qb[:, 2:4] in _gn_fold,
        # so the V path needs no bias accumulate on the PE at all)

        # --- QKV projections.  All PSUM->SBUF drains alternate ACT/DVE
        # (GPSIMD has no PSUM port).
        cp_i = [0]

        def drain(dst, src, bias_col=None):
            a_turn = cp_i[0] % 2 == 0
            cp_i[0] += 1
            if a_turn:
                nc.scalar.activation(out=dst, in_=src, func=AF.Identity,
                                     bias=0.0 if bias_col is None else bias_col)
            elif bias_col is None:
                nc.vector.tensor_copy(out=dst, in_=src)
            else:
                nc.vector.tensor_scalar_add(out=dst, in0=src, scalar1=bias_col)

        def q_block(nb):
            for pr in range(2):
                st = ps_st.tile([128, 512], F32, tag="st")
                nc.tensor.matmul(st,
                                 lhsT=w8[:, :, 128 * pr:128 * (pr + 1)],
                                 rhs=x8[:, :, 512 * nb:512 * (nb + 1)],
                                 start=True, stop=True, perf_mode=DR)
                drain(q_pair[pr][:, 512 * nb:512 * (nb + 1)],
                      st, qb[:, pr:pr + 1])

        def k_block(nb):
            # k bias is dropped: softmax is invariant to per-query constants
            for pr in range(2):
                st = ps_st.tile([128, 512], F32, tag="st")
                nc.tensor.matmul(st,
                                 lhsT=w8[:, :, 256 + 128 * pr:256 + 128 * (pr + 1)],
                                 rhs=x8[:, :, 512 * nb:512 * (nb + 1)],
                                 start=True, stop=True, perf_mode=DR)
                drain(k_all[:, pr, 0, 512 * nb:512 * (nb + 1)], st)

        def v_group(g):
            # two 128-position blocks (256 v-channels each) per 512-col tile
            st = ps_st.tile([128, 512], F32, tag="st")
            for i in range(2):
                b = 2 * g + i
                nc.tensor.matmul(st[:, 256 * i:256 * (i + 1)],
                                 lhsT=x8[:, :, 128 * b:128 * (b + 1)],
                                 rhs=w8[:, :, 512:768],
                                 start=True, stop=True, perf_mode=DR)
            drain(vt_sb[:, 2 * g:2 * g + 2, :, 0:64],
                  st.rearrange("p (j h d) -> p j h d", j=2, d=64))

        if do_front:
            for nb in range(4):
                q_block(nb)
                k_block(nb)
                v_group(2 * nb)
                v_group(2 * nb + 1)
            for nb in range(4, 8):
                k_block(nb)
                v_group(2 * nb)
                v_group(2 * nb + 1)

        # --- attention
        def make_proj(ib, at2):
            def proj():
                sts = []
                for ob in range(2):
                    stx = ps_st.tile([128, 512], F32, tag="st")
                    sts.append(stx)
                    nc.tensor.matmul(stx,
                                     lhsT=wp8[:, :, 128 * ob:128 * (ob + 1)],
                                     rhs=at2, start=True, stop=False,
                                     perf_mode=DR)
                    # residual: st += 32 * x  (f32 identity matmul)
                    nc.tensor.matmul(
                        stx, lhsT=ident32,
                        rhs=x_t[:, ob, 512 * ib:512 * (ib + 1)],
                        start=False, stop=True)
                for ob in range(2):
                    ot = work.tile([128, 512], F32, tag="ot")
                    nc.scalar.activation(out=ot, in_=sts[ob],
                                         func=AF.Identity, scale=PDEN,
                                         bias=qb[:, 2 + ob:3 + ob])
                    nc.sync.dma_start(
                        out=out_r[:, ob, 512 * ib:512 * (ib + 1)], in_=ot)
            return proj

        if not do_attn:
            for ib in range(4):
                at2 = atp.tile([128, 2, 512], FP8, tag="at2")
                nc.gpsimd.memset(at2, 0.015625)
                make_proj(ib, at2)()
            return

        noav = "noav" in PROBE
        noscores = "noscores" in PROBE
        densepe = "densepe" in PROBE
        pt4 = None
        if noscores or densepe:
            pt4 = []
            for i in range(4):
                t_ = ptp.tile([128, 2, 512], FP8, tag="pt", name="pt")
                nc.gpsimd.memset(t_, 0.0078125)
                pt4.append(t_)

        pend_proj = None
        pend_norm = None
        for ib in range(4):
            at2 = atp.tile([128, 2, 512], FP8, tag="at2")
            for h in range(4):
                av = ps_av.tile([65, 512], F32, tag="av")
                pend = []

                def sc_pair(t, h=h, ib=ib):
                    if noscores:
                        return pt4[t % 4]
                    pt = ptp.tile([128, 2, 512], FP8, tag="pt", name="pt")
                    hh = h % 2
                    for u in range(2):
                        kb = 2 * t + u
                        st = ps_st.tile([128, 512], F32, tag="st")
                        if SC_DR:
                            nc.tensor.matmul(
                                st,
                                lhsT=k_all[64 * hh:64 * (hh + 1), h // 2, :,
                                           128 * kb:128 * (kb + 1)],
                                rhs=q_bcast(h, ib),
                                start=True, stop=True, perf_mode=DR)
                        else:
                            nc.tensor.matmul(
                                st,
                                lhsT=k_all[64 * hh:64 * (hh + 1), h // 2, 0,
                                           128 * kb:128 * (kb + 1)],
                                rhs=q_pair[h // 2][
                                    64 * hh:64 * (hh + 1),
                                    512 * ib:512 * (ib + 1)],
                                start=True, stop=True)
                        if EXP_PAT[kb] == "A":
                            nc.scalar.activation(out=pt[:, u], in_=st,
                                                 func=AF.Exp,
                                                 scale=ATT_SCALE)
                        else:
                            nc.vector.tensor_scalar(
                                out=pt[:, u].bitcast(U8), in0=st,
                                scalar1=EC1, scalar2=EC2,
                                op0=OP.mult, op1=OP.add)
                    return pt

                def av_step(t, pt, av=av, h=h):
                    if noav:
                        return
                    if densepe:
                        pt = pt4[t % 4]
                    for r in range(AV_REPEAT):
                        nc.tensor.matmul(
                            av, lhsT=vt_sb[:, 2 * t:2 * t + 2, h, 0:65],
                            rhs=pt,
                            start=(t == 0 and r == 0),
                            stop=(t == 15 and r == AV_REPEAT - 1),
                            perf_mode=DR)

                for t in range(16):
                    pend.append((t, sc_pair(t)))
                    if t == 4 and pend_norm is not None:
                        pend_norm()
                        pend_norm = None
                    if t == 13 and pend_proj is not None:
                        pend_proj()
                        pend_proj = None
                    if t >= AV_DEFER and (t - AV_DEFER) % AV_GROUP == \
                            AV_GROUP - 1:
                        for _ in range(AV_GROUP):
                            av_step(*pend.pop(0))
                while pend:
                    av_step(*pend.pop(0))
                if noav:
                    continue

                # normalize: at2[d, i] = av[d, i] * (32 / den[i]).  The
                # reciprocal + broadcast issue now; the multiply is deferred
                # into the next head's window to hide the broadcast latency.
                rec_b = work.tile([65, 512], BF16, tag="rec_b")
                with nc.allow_low_precision(reason="bf16 softmax denom"):
                    nc.vector.reciprocal(out=rec_b[64:65], in_=av[64:65])
                last = ib == 3 and h == 3
                if NORM_MODE == "pe" or last:
                    # broadcast on the PE (outer product into a spare av
                    # bank) and stage through SBUF for the multiply: no DMA
                    # and no 900ns DMA-completion semaphore in the window
                    # steady state.
                    rbq = ps_av.tile([65, 512], F32, tag="av", name="rbq")
                    nc.tensor.matmul(rbq[0:64], lhsT=ones64b[64:65, :],
                                     rhs=rec_b[64:65], start=True, stop=True)

                    def mult(av=av, rbq=rbq, h=h, at2=at2):
                        rbs = work.tile([64, 512], BF16, tag="rbs")
                        nc.vector.tensor_copy(out=rbs, in_=rbq[0:64])
                        nc.vector.tensor_mul(
                            out=at2[64 * (h % 2):64 * (h % 2) + 64, h // 2],
                            in0=av[0:64], in1=rbs)
                    if last:
                        mult()
                        break
                else:
                    rb = rbp.tile([64, 512], BF16, tag="rb")
                    rsrc = rec_b[64:65]
                    nc.sync.dma_start(out=rb, in_=bass.AP(
                        tensor=rsrc.tensor, offset=rsrc.offset,
                        ap=[list(rsrc.ap[0]), [0, 64]] + list(rsrc.ap[1:])))

                    def mult(av=av, rb=rb, h=h, at2=at2):
                        nc.vector.tensor_mul(
                            out=at2[64 * (h % 2):64 * (h % 2) + 64, h // 2],
                            in0=av[0:64], in1=rb)
                pend_norm = mult

            if not noav:
                pend_proj = make_proj(ib, at2)
        if pend_norm is not None:
            pend_norm()
            pend_norm = None
        if pend_proj is not None:
            pend_proj()


def build_program(split_waits=True, iters=1, mode="full"):
    _apply_drain_patch()
    nc = bass.Bass()
    xbf = nc.declare_dram_parameter("x_bf16", [C, N], BF16, isOutput=False)
    xf8 = nc.declare_dram_parameter("x_fp8", [C, N], FP8, isOutput=False)
    wqkv = nc.declare_dram_parameter("w_qkvT", [C, 3 * C], F32, isOutput=False)
    wproj = nc.declare_dram_parameter("w_projT", [C, C], F32, isOutput=False)
    gam = nc.declare_dram_parameter("gn_gamma", [C], F32, isOutput=False)
    bet = nc.declare_dram_parameter("gn_beta", [C], F32, isOutput=False)
    bpr = nc.declare_dram_parameter("b_proj", [C], F32, isOutput=False)
    inda = nc.declare_dram_parameter("indA", [C, 8], F32, isOutput=False)
    indb = nc.declare_dram_parameter("indB", [8, C], F32, isOutput=False)
    zr8 = nc.declare_dram_parameter("zeros8", [128, N], mybir.dt.uint8,
                                    isOutput=False)
    out = nc.declare_dram_parameter("out", [C, NQ], F32, isOutput=True)
    with tile.TileContext(nc) as tc:
        for _ in range(iters):
            _body(tc, xbf, xf8, wqkv, wproj, gam, bet, bpr, inda, indb, zr8,
                  out, mode=mode)
    if split_waits:
        _split_excess_waits(nc)
    return nc


def make_in_maps(x, gn_gamma, gn_beta, w_qkv, w_proj, b_proj):
    x = np.ascontiguousarray(x, dtype=np.float32)
    w_qkvT = np.ascontiguousarray(np.asarray(w_qkv, np.float32).T)
    w_projT = np.ascontiguousarray(np.asarray(w_proj, np.float32).T)
    gn_gamma = np.ascontiguousarray(gn_gamma, dtype=np.float32)
    gn_beta = np.ascontiguousarray(gn_beta, dtype=np.float32)
    b_proj = np.ascontiguousarray(b_proj, dtype=np.float32)
    ch_groups = np.arange(C) // 32
    indA = np.zeros((C, 8), np.float32)
    indA[np.arange(C), ch_groups] = 1.0 / 32.0
    indB = np.zeros((8, C), np.float32)
    indB[ch_groups, np.arange(C)] = 1.0
    in_maps = []
    bf16_np = mybir.dt.np(BF16)
    fp8_np = mybir.dt.np(FP8)
    for core in range(NCORES):
        s, half = core // 2, core % 2
        xfl = x[s].reshape(C, N)
        x_core = np.ascontiguousarray(np.concatenate(
            [xfl[:, half * NQ:(half + 1) * NQ],
             xfl[:, (1 - half) * NQ:(2 - half) * NQ]], axis=1))
        in_maps.append({
            "x_bf16": x_core.astype(bf16_np),
            "x_fp8": x_core.astype(fp8_np),
            "w_qkvT": w_qkvT,
            "w_projT": w_projT,
            "gn_gamma": gn_gamma,
            "gn_beta": gn_beta,
            "b_proj": b_proj,
            "indA": indA,
            "indB": indB,
            "zeros8": np.zeros((128, N), np.uint8),
        })
    return in_maps


def assemble_output(results):
    out = np.empty((B, C, N), np.float32)
    for core in range(NCORES):
        s, half = core // 2, core % 2
        out[s][:, half * NQ:(half + 1) * NQ] = results[core]["out"]
    return out.reshape(B, C, HGT, WID)


_PROGRAM_CACHE = {}


def kernel(x, gn_gamma, gn_beta, w_qkv, w_proj, b_proj):
    if "nc" not in _PROGRAM_CACHE:
        _PROGRAM_CACHE["nc"] = build_program()
    nc = _PROGRAM_CACHE["nc"]
    in_maps = make_in_maps(x, gn_gamma, gn_beta, w_qkv, w_proj, b_proj)
    res = run_bass_kernel_spmd(nc, in_maps, list(range(NCORES)))
    return assemble_output(res.results)



# revision 61
# speedup vs baseline: 1.1794x; 1.1671x over previous
"""AttentionBlock (GroupNorm + 1x1-conv QKV + MHSA + proj + residual) on 8
Trainium2 NeuronCores via Bass/Tile.

Sharding: 8 cores = 4 samples x 2 query-halves. The host reorders each
sample's spatial columns so the core's query half occupies columns 0:2048;
keys/values cover all 4096 columns (attention is permutation-invariant over
keys). Each core computes GroupNorm statistics + full K/V, Q for its 2048
queries, attention, projection + residual for its half. No collectives.

Numerics: all attention-path matmuls run in fp8e4 with the DoubleRow perf
mode (0.5 cycles/row, two stacked contraction slots per step):
  - QKV/proj contract 256 channels as [128, 2, *]
  - scores per head contract dh=64 at partition base 64*(h%2) in head-pair
    tiles (k slot 1 is zeroed, q is read through a 0-stride broadcast AP,
    so the DoubleRow second slot contributes nothing)
  - AV contracts 256 keys (two 128-key blocks) per step
Softmax exp drains the score PSUM through the only two engines with a PSUM
port (GPSIMD has none on TRN2): ACT runs the real Exp into fp8e4, DVE runs
a Schraudolph bit-trick exp (u8 = s*c1 + c2 truncated, bitcast as fp8e4),
17/15 interleaved.  GPSIMD carries the SBUF-side work (x/w fp8 converts).
The fp8 ones-column in vT holds 1/32 so the softmax denominator stays in
fp8 range; the 32x is folded back in the output projection epilogue, and
the residual (+32x, exact f32 identity matmul) is accumulated into the
projection PSUM by the PE.  GroupNorm is folded into the QKV weights
(W' = W * a[c], bias = W^T b) so normalized activations never materialize;
the k bias is dropped (softmax is invariant to per-query constants), and
the v bias is folded through W_proj into the projection bias
(out = W_p(AV + b_v 1^T) + b_p = W_p AV + (W_p b_v + b_p)), removing all
32 ones-row accumulates from the V path.  The
per-query reciprocal row is broadcast across 64 partitions by an SP-queue
DMA whose latency is hidden by deferring the normalize-multiply into the
next head's window.  The attention path's absolute accuracy is relaxed
(fp8 + approx exp), which is safe here: the residual dominates the output
norm by ~60x.

All PSUM traffic is organized as [128, 512] single-bank tiles with a
6-deep score rotation (6 score banks + 2 AV banks = the full 16KB):
on hardware the PSUM write-after-write semaphore round-trip costs ~450ns
per reuse (measured; CoreSim models ~100ns), so shallow rotations gate
the PE far below its streaming rate.  CoreSim cost model: 218us/core;
measured on TRN2 (marginal per-iteration of a multi-pass NEFF, tunnel
dispatch overhead cancelled): ~460-505us/core.

The residual HW-vs-sim factor appears only when score+exp and
AV+normalize run concurrently (HW A/B: scores+exp alone 193us, AV side
alone 119us, both ~500us).  The av2x probe (every AV step accumulated
twice -- numerically identity since numerator and denominator scale
together) showed each AV matmul costs ~390ns SERIALLY on HW vs the
~107ns model; issuing AV steps in back-to-back pairs (AV_GROUP=2, with
PTP_BUFS=6) recovers ~45us, consistent with a PSUM accumulate-bank
reopen cost on the PE.  Falsified by direct HW experiment: pt/rb pool
depth alone, AV issue deferral (2/4/8/16), score-PSUM grain (3x1024 vs
6x512), 5+3 PSUM split, exp engine assignment (all-ACT/all-DVE/blocked),
PE-broadcast vs DMA-broadcast normalization, PE p-state keep-warm.
Best measured config is encoded in the module defaults below
(HW ~375-425us by the marginal-iteration protocol; CoreSim 212us).
"""

import numpy as np

import concourse.bass as bass
import concourse.tile as tile
from concourse import mybir
from concourse.bass_utils import run_bass_kernel_spmd
from concourse.masks import make_identity
from concourse.tile import ScopedClock

# ---------------------------------------------------------------- constants
B, C, HGT, WID = 4, 256, 64, 64
N = HGT * WID            # 4096 spatial positions
NQ = N // 2              # query half per core
HEADS = 4
DH = C // HEADS          # 64
EPS = 1e-5
ATT_SCALE = (C * HEADS) ** (-0.5)   # 1/32
NCORES = 8

F32 = mybir.dt.float32
F32R = mybir.dt.float32r
BF16 = mybir.dt.bfloat16
FP8 = mybir.dt.float8e4
U8 = mybir.dt.uint8
DR = mybir.MatmulPerfMode.DoubleRow

# Schraudolph exp in fp8e4 bit-space: bits(exp(s*ATT_SCALE)) ~= s*EC1 + EC2
LOG2E = 1.4426950408889634
EC1 = ATT_SCALE * LOG2E * 8.0
EC2 = 56.06
PDEN = 1.0 / 32.0        # ones-column value; folded back in the epilogue

# per-(head, query-block) engine schedule for the 32 exp blocks of a
# window.  GPSIMD has no PSUM port on TRN2, so only ACT (A) and DVE (D)
# can read the score PSUM; 17A/15D balances their HW rates.
EXP_PAT = "AD" * 15 + "AA"

# probe knobs (timing experiments only): "noav" skips AV/normalize/proj,
# "noscores" feeds AV from 4 pre-memset pt tiles instead of scores+exp.
PROBE = ""
PTP_BUFS = 6
RBP_BUFS = 3
AV_DEFER = 2     # how many sc tiles ahead of each av step (16 = full window)
AV_GROUP = 2     # issue av steps in back-to-back pairs: halves the PE's
                 # PSUM accumulate-bank reopens (measured ~45us on HW;
                 # groups of 4 are worse -- latency outweighs the saving)
ST_BUFS = 6      # score PSUM banks; ST_BUFS + AV_BUFS <= 8 (16KB PSUM)
AV_BUFS = 2      # AV PSUM banks
NORM_MODE = "dma"  # "pe": broadcast 1/den via PE outer product; "dma":
                   # SP-queue broadcast DMA (measured fastest on HW)
SC_DR = True     # scores in DoubleRow (0.5 cyc/row) vs plain fp8 (1 cyc)
AV_REPEAT = 1    # accumulate each AV step this many times: numerator and
                 # denominator scale together, so the softmax output is
                 # unchanged -- pure PE keep-warm work (p-state probe)

# ------------------------------------------------- walrus multi-wait patch
# The external neuronxcc walrus rejects >2 sync waits on a CTRL (Drain)
# instruction; split the Tile exit-clock waits across nofuse sync NOPs.
_MAXW = 1


def _split_drain_and_barrier(self, tick_clock, wait_clock):
    nc = self.nc
    probe = nc.sync.nop(nofuse=True, hint="drain_wait_probe")
    wait_clock.add_sem_waits(probe.ins, ScopedClock({None: tick_clock.global_clock}))
    si = probe.ins.sync_info
    waits = list(si.on_wait) if si is not None else []
    probe.ins.sync_info = mybir.SyncInfo(on_wait=waits[:_MAXW], on_update=[])
    rest = waits[_MAXW:]
    for i in range(0, len(rest), _MAXW):
        nop = nc.sync.nop(nofuse=True, hint=f"drain_wait_{i}")
        nop.ins.sync_info = mybir.SyncInfo(on_wait=rest[i:i + _MAXW], on_update=[])
    nc.sync.drain()
    nc.all_engine_barrier()
    assert self.sems is not None
    popped = nc._tile_sem_poison_stack.pop()
    assert popped is self._sem_poison
    nc.clear_and_free_semaphores(list(self.sems.allocated().values()))
    nc.all_engine_barrier()


def _apply_drain_patch():
    tile.TileContext._drain_and_barrier = _split_drain_and_barrier


def _split_excess_waits(nc):
    """External walrus accepts only one sync wait per instruction; hoist
    excess waits onto same-engine nofuse NOPs inserted just before."""
    k = 0
    for bb in nc.m.functions[0].blocks:
        insts = bb.instructions
        i = 0
        while i < len(insts):
            inst = insts[i]
            si = inst.sync_info
            if si is not None and len(si.on_wait) > 1:
                waits = list(si.on_wait)
                inst.sync_info = mybir.SyncInfo(on_wait=waits[-1:],
                                                on_update=list(si.on_update))
                nops = []
                for w in waits[:-1]:
                    nop = mybir.InstNoOp(
                        name=f"I-wsplit{k}",
                        sync_info=mybir.SyncInfo(on_wait=[w], on_update=[]),
                        bass_nofuse=True,
                        engine=inst.engine,
                    )
                    k += 1
                    nops.append(nop)
                insts[i:i] = nops
                bb.instructions = insts
                i += len(nops)
            i += 1
    return k


# ------------------------------------------------------------- the program
def _body(tc, xbf, xf8, wqkv, wproj, gam, bet, bpr, inda, indb, zr8, out,
          mode="full"):
    nc = tc.nc
    AF = mybir.ActivationFunctionType
    OP = mybir.AluOpType
    do_front = mode in ("full", "noattn")
    do_attn = mode in ("full", "attnonly")

    persist_cm = tc.tile_pool(name="persist", bufs=1)
    work_cm = tc.tile_pool(name="work", bufs=2)
    persist = persist_cm.__enter__()
    work = work_cm.__enter__()

    # ---------------- load inputs (x split across 3 DMA queues).  x ships
    # from the host pre-converted: bf16 for GN stats + residual, fp8 for
    # the QKV matmuls (no on-device converts, half the f32 DMA bytes).
    x_t = persist.tile([128, 2, N], BF16)
    xbf_r = xbf.rearrange("(o p) n -> p o n", p=128)
    # chunk 0 is split in two so bn_stats can start one DMA-latency earlier
    x_chunks = [(0, 256), (256, 256)] + [(512 * c, 512) for c in range(1, 8)]
    chunk_eng = [nc.sync, nc.sync, nc.scalar, nc.sync, nc.scalar,
                 nc.sync, nc.scalar, nc.sync, nc.scalar]
    w_t = persist.tile([128, 2, 3 * C], F32)
    wp_t = persist.tile([128, 2, C], F32)
    gam_t = persist.tile([128, 2], F32)
    bet_t = persist.tile([128, 2], F32)
    bpr_t = persist.tile([128, 2], F32)
    x8 = persist.tile([128, 2, N], FP8)
    w8 = persist.tile([128, 2, 3 * C], FP8)
    wp8 = persist.tile([128, 2, C], FP8)
    if do_front:
        for (off, sz), eng in zip(x_chunks, chunk_eng):
            eng.dma_start(out=x_t[:, :, off:off + sz],
                          in_=xbf_r[:, :, off:off + sz])
        nc.scalar.dma_start(out=x8,
                            in_=xf8.rearrange("(o p) n -> p o n", p=128))
        nc.sync.dma_start(out=w_t,
                          in_=wqkv.rearrange("(o p) m -> p o m", p=128))
        nc.sync.dma_start(out=wp_t,
                          in_=wproj.rearrange("(o p) m -> p o m", p=128))
        nc.sync.dma_start(out=gam_t, in_=gam.rearrange("(o p) -> p o", p=128))
        nc.sync.dma_start(out=bet_t, in_=bet.rearrange("(o p) -> p o", p=128))
        nc.sync.dma_start(out=bpr_t, in_=bpr.rearrange("(o p) -> p o", p=128))
    out_r = out.rearrange("(o p) n -> p o n", p=128)

    # ---------------- phase 1: GroupNorm stats -> per-channel affine (a, b)
    qb = persist.tile([128, 6], F32)
    with tc.tile_pool(name="ps_small", bufs=1, space="PSUM") as ps_small:
        ident = persist.tile([128, 128], F32)
        make_identity(nc, ident)
        # bf16: 32.0 and the 0/32 entries are exact, and the residual
        # matmul runs at 1 cyc/row instead of f32's 4
        ident32 = persist.tile([128, 128], BF16)
        nc.vector.tensor_scalar_mul(out=ident32, in0=ident, scalar1=32.0)
        ones1b = persist.tile([1, 128], BF16)
        nc.vector.memset(ones1b, 1.0)
        if do_front:
            _gn_fold(tc, persist, work, ps_small, x_t, x_chunks, w_t, wp_t,
                     gam_t, bet_t, bpr_t, inda, indb, qb, x8, w8, wp8)
    _qkv_attn(tc, persist, work, x_t, x8, w8, wp8, qb, bpr_t, out_r, zr8,
              ident, ident32, ones1b, do_front, do_attn)

    work_cm.__exit__(None, None, None)
    persist_cm.__exit__(None, None, None)


def _gn_fold(tc, persist, work, ps_small, x_t, x_chunks, w_t, wp_t,
             gam_t, bet_t, bpr_t, inda, indb, qb, x8, w8, wp8):
    nc = tc.nc
    AF = mybir.ActivationFunctionType
    OP = mybir.AluOpType
    if True:
        # pre-warm the ACT sqrt table while DMAs run so the GN-path Sqrt
        # doesn't pay the 1.3us table load
        eps_t = persist.tile([8, 1], F32)
        nc.vector.memset(eps_t, EPS)
        warm = work.tile([8, 1], F32, tag="warm")
        nc.scalar.activation(out=warm, in_=eps_t, func=AF.Sqrt)

        # bn_stats on DVE for both channel halves (x8 ships pre-converted)
        stats6 = work.tile([128, 2, 9, 6], F32, tag="stats6")
        mv = work.tile([128, 2, 2], F32, tag="mv")
        for ch in range(2):
            for s, (off, sz) in enumerate(x_chunks):
                nc.vector.bn_stats(out=stats6[:, ch, s],
                                   in_=x_t[:, ch, off:off + sz])
            nc.vector.bn_aggr(out=mv[:, ch], in_=stats6[:, ch])
        nc.gpsimd.tensor_copy(out=wp8, in_=wp_t)
        # per-channel (mean, E[x^2])
        st2 = work.tile([128, 2, 2], F32, tag="st2")
        msq = work.tile([128, 1], F32, tag="msq")
        for ch in range(2):
            nc.vector.tensor_copy(out=st2[:, ch, 0:1], in_=mv[:, ch, 0:1])
            nc.vector.tensor_mul(out=msq, in0=mv[:, ch, 0:1], in1=mv[:, ch, 0:1])
            nc.vector.tensor_add(out=st2[:, ch, 1:2], in0=mv[:, ch, 1:2], in1=msq)

        # group reduce across channels: indicator matmul, values 1/32
        indA = persist.tile([128, 2, 8], F32)
        nc.sync.dma_start(out=indA, in_=inda.rearrange("(o p) g -> p o g", p=128))
        gps = ps_small.tile([128, 8], F32, tag="gps")
        for ch in range(2):
            nc.tensor.matmul(gps[0:8, 0:2], lhsT=indA[:, ch], rhs=st2[:, ch],
                             start=(ch == 0), stop=(ch == 1))
        # group var -> rstd;  gw cols: 0 mean, 1 rstd, 2 mean-work, 3 var-work
        gw = persist.tile([8, 4], F32)
        nc.vector.tensor_copy(out=gw[:, 2:4], in_=gps[0:8, 0:2])
        nc.vector.tensor_copy(out=gw[:, 0:1], in_=gw[:, 2:3])
        gmsq = work.tile([8, 1], F32, tag="gmsq")
        nc.vector.tensor_mul(out=gmsq, in0=gw[:, 2:3], in1=gw[:, 2:3])
        nc.vector.tensor_tensor(out=gw[:, 3:4], in0=gw[:, 3:4], in1=gmsq,
                                op=OP.subtract)
        nc.scalar.activation(out=gw[:, 3:4], in_=gw[:, 3:4], func=AF.Sqrt,
                             bias=eps_t)
        nc.vector.reciprocal(out=gw[:, 1:2], in_=gw[:, 3:4])

        # broadcast group (mean, rstd) back to channels
        indB = persist.tile([8, 2, 128], F32)
        nc.sync.dma_start(out=indB, in_=indb.rearrange("g (o p) -> g o p", p=128))
        chst = persist.tile([128, 2, 2], F32)   # [p, ch, {mean, rstd}]
        for ch in range(2):
            cp = ps_small.tile([128, 2], F32, tag="chps")
            nc.tensor.matmul(cp, lhsT=indB[:, ch], rhs=gw[:, 0:2],
                             start=True, stop=True)
            nc.vector.tensor_copy(out=chst[:, ch], in_=cp)

        # a = rstd * gamma ; b = beta - mean * a
        ab = persist.tile([128, 2, 2], F32)     # [p, ch, {a, b}]
        abt = work.tile([128, 1], F32, tag="abt")
        for ch in range(2):
            nc.vector.tensor_mul(out=ab[:, ch, 0:1], in0=chst[:, ch, 1:2],
                                 in1=gam_t[:, ch:ch + 1])
            nc.vector.tensor_mul(out=abt, in0=chst[:, ch, 0:1],
                                 in1=ab[:, ch, 0:1])
            nc.vector.tensor_tensor(out=ab[:, ch, 1:2], in0=bet_t[:, ch:ch + 1],
                                    in1=abt, op=OP.subtract)

        # ---------------- phase 2: fold GN into weights
        # qkv_bias[o] = sum_c W[o, c] * b[c]   (original W).  The k bias
        # (ob 2, 3) is unused: softmax is invariant to per-query constants.
        qbp = ps_small.tile([128, 8], F32, tag="qbp")
        for ob in (0, 1, 4, 5):
            for ch in range(2):
                nc.tensor.matmul(qbp[:, ob:ob + 1],
                                 lhsT=w_t[:, ch, 128 * ob:128 * (ob + 1)],
                                 rhs=ab[:, ch, 1:2],
                                 start=(ch == 0), stop=(ch == 1))
        nc.vector.tensor_copy(out=qb[:, 0:2], in_=qbp[:, 0:2])
        nc.vector.tensor_copy(out=qb[:, 4:6], in_=qbp[:, 4:6])
        # fold the v-bias through W_proj into the projection bias:
        # out = W_p(AV + b_v 1^T) + b_p = W_p AV + (W_p b_v + b_p).
        # qb cols 2:4 hold the combined projection bias; the V path then
        # needs no ones-row accumulate at all.
        wpp = ps_small.tile([128, 2], F32, tag="wpp")
        for ob in range(2):
            for ch in range(2):
                nc.tensor.matmul(wpp[:, ob:ob + 1],
                                 lhsT=wp_t[:, ch, 128 * ob:128 * (ob + 1)],
                                 rhs=qb[:, 4 + ch:5 + ch],
                                 start=(ch == 0), stop=(ch == 1))
        for ob in range(2):
            nc.vector.tensor_add(out=qb[:, 2 + ob:3 + ob],
                                 in0=wpp[:, ob:ob + 1],
                                 in1=bpr_t[:, ob:ob + 1])
        # W'[c, o] = W[c, o] * a[c], fused with the fp8 conversion
        # (q section on DVE so Q matmuls start first)
        for ch in range(2):
            nc.vector.tensor_scalar_mul(out=w8[:, ch, 0:256],
                                        in0=w_t[:, ch, 0:256],
                                        scalar1=ab[:, ch, 0:1])
            nc.gpsimd.tensor_scalar_mul(out=w8[:, ch, 256:768],
                                        in0=w_t[:, ch, 256:768],
                                        scalar1=ab[:, ch, 0:1])


def _qkv_attn(tc, persist, work, x_t, x8, w8, wp8, qb, bpr_t, out_r, zr8,
              ident, ident32, ones1b, do_front, do_attn):
    nc = tc.nc
    AF = mybir.ActivationFunctionType
    OP = mybir.AluOpType

    # ---------------- phase 3: QKV + attention, all fp8 DoubleRow
    # q/k head-pair tiles: partition = 64*(h%2) + d, tile index = h//2.
    # k slot 1 is zero so the scores DoubleRow second slot is inert.
    q_pair = [persist.tile([128, NQ], FP8, name=f"q_pair{i}")
              for i in range(2)]
    # k layout [p, head-pair, slot, n]; slot 1 is zero (DoubleRow inert slot)
    k_all = persist.tile([128, 2, 2, N], FP8)
    for pr in range(2):
        nc.sync.dma_start(out=k_all[:, pr, 1].bitcast(U8), in_=zr8[:, :])
    # vT layout [pos, key-block, head, 128]: DoubleRow ldweights requires the
    # slot-pair stride to be 128-aligned, so each head's 65 columns (64 dims
    # + the 1/32 denominator column) sit in their own 128-wide slot.
    vt_sb = persist.tile([128, 32, 4, 128], FP8)
    nc.vector.memset(vt_sb[:, :, :, 64:65], PDEN)
    ones64b = persist.tile([128, 64], BF16)
    nc.vector.memset(ones64b, 1.0)
    if not do_front:
        # attnonly probe: initialize everything the attention windows read
        # (idle engines only: SP-queue DMAs + Pool memsets)
        for i in range(2):
            nc.sync.dma_start(out=q_pair[i].bitcast(U8), in_=zr8[:, 0:NQ])
        for pr in range(2):
            nc.sync.dma_start(out=k_all[:, pr, 0].bitcast(U8), in_=zr8[:, :])
        nc.gpsimd.memset(vt_sb[:, :, :, 0:64], 0.015625)
        nc.gpsimd.memset(x_t, 0.5)
        nc.gpsimd.memset(wp8, 0.015625)
        nc.gpsimd.memset(bpr_t, 0.0)
        nc.gpsimd.memset(qb, 0.0)

    def q_bcast(h, ib):
        base = q_pair[h // 2][64 * (h % 2):64 * (h % 2) + 64,
                              512 * ib:512 * (ib + 1)]
        return bass.AP(tensor=base.tensor, offset=base.offset,
                       ap=[base.ap[0], [0, 2], base.ap[1]])

    with (
        tc.tile_pool(name="ps_st", bufs=ST_BUFS, space="PSUM") as ps_st,
        tc.tile_pool(name="ps_av", bufs=AV_BUFS, space="PSUM") as ps_av,
        tc.tile_pool(name="ptp", bufs=PTP_BUFS) as ptp,
        tc.tile_pool(name="atp", bufs=2) as atp,
        tc.tile_pool(name="rbp", bufs=RBP_BUFS) as rbp,
    ):
        # (v-bias is folded through W_proj into qb[:, 2:4] in _gn_fold,
        # so the V path needs no bias accumulate on the PE at all)

        # --- QKV projections.  All PSUM->SBUF drains alternate ACT/DVE
        # (GPSIMD has no PSUM port).
        cp_i = [0]

        def drain(dst, src, bias_col=None):
            a_turn = cp_i[0] % 2 == 0
            cp_i[0] += 1
            if a_turn:
                nc.scalar.activation(out=dst, in_=src, func=AF.Identity,
                                     bias=0.0 if bias_col is None else bias_col)
            elif bias_col is None:
                nc.vector.tensor_copy(out=dst, in_=src)
            else:
                nc.vector.tensor_scalar_add(out=dst, in0=src, scalar1=bias_col)

        def q_block(nb):
            for pr in range(2):
                st = ps_st.tile([128, 512], F32, tag="st")
                nc.tensor.matmul(st,
                                 lhsT=w8[:, :, 128 * pr:128 * (pr + 1)],
                                 rhs=x8[:, :, 512 * nb:512 * (nb + 1)],
                                 start=True, stop=True, perf_mode=DR)
                drain(q_pair[pr][:, 512 * nb:512 * (nb + 1)],
                      st, qb[:, pr:pr + 1])

        def k_block(nb):
            # k bias is dropped: softmax is invariant to per-query constants
            for pr in range(2):
                st = ps_st.tile([128, 512], F32, tag="st")
                nc.tensor.matmul(st,
                                 lhsT=w8[:, :, 256 + 128 * pr:256 + 128 * (pr + 1)],
                                 rhs=x8[:, :, 512 * nb:512 * (nb + 1)],
                                 start=True, stop=True, perf_mode=DR)
                drain(k_all[:, pr, 0, 512 * nb:512 * (nb + 1)], st)

        def v_group(g):
            # two 128-position blocks (256 v-channels each) per 512-col tile
            st = ps_st.tile([128, 512], F32, tag="st")
            for i in range(2):
                b = 2 * g + i
                nc.tensor.matmul(st[:, 256 * i:256 * (i + 1)],
                                 lhsT=x8[:, :, 128 * b:128 * (b + 1)],
                                 rhs=w8[:, :, 512:768],
                                 start=True, stop=True, perf_mode=DR)
            drain(vt_sb[:, 2 * g:2 * g + 2, :, 0:64],
                  st.rearrange("p (j h d) -> p j h d", j=2, d=64))

        if do_front:
            for nb in range(4):
                q_block(nb)
                k_block(nb)
                v_group(2 * nb)
                v_group(2 * nb + 1)
            for nb in range(4, 8):
                k_block(nb)
                v_group(2 * nb)
                v_group(2 * nb + 1)

        # --- attention
        def make_proj(ib, at2):
            def proj():
                sts = []
                for ob in range(2):
                    stx = ps_st.tile([128, 512], F32, tag="st")
                    sts.append(stx)
                    nc.tensor.matmul(stx,
                                     lhsT=wp8[:, :, 128 * ob:128 * (ob + 1)],
                                     rhs=at2, start=True, stop=False,
                                     perf_mode=DR)
                    # residual: st += 32 * x  (f32 identity matmul)
                    nc.tensor.matmul(
                        stx, lhsT=ident32,
                        rhs=x_t[:, ob, 512 * ib:512 * (ib + 1)],
                        start=False, stop=True)
                for ob in range(2):
                    ot = work.tile([128, 512], F32, tag="ot")
                    nc.scalar.activation(out=ot, in_=sts[ob],
                                         func=AF.Identity, scale=PDEN,
                                         bias=qb[:, 2 + ob:3 + ob])
                    nc.sync.dma_start(
                        out=out_r[:, ob, 512 * ib:512 * (ib + 1)], in_=ot)
            return proj

        if not do_attn:
            for ib in range(4):
                at2 = atp.tile([128, 2, 512], FP8, tag="at2")
                nc.gpsimd.memset(at2, 0.015625)
                make_proj(ib, at2)()
            return

        noav = "noav" in PROBE
        noscores = "noscores" in PROBE
        densepe = "densepe" in PROBE
        pt4 = None
        if noscores or densepe:
            pt4 = []
            for i in range(4):
                t_ = ptp.tile([128, 2, 512], FP8, tag="pt", name="pt")
                nc.gpsimd.memset(t_, 0.0078125)
                pt4.append(t_)

        pend_proj = None
        pend_norm = None
        for ib in range(4):
            at2 = atp.tile([128, 2, 512], FP8, tag="at2")
            for h in range(4):
                av = ps_av.tile([65, 512], F32, tag="av")
                pend = []

                def sc_pair(t, h=h, ib=ib):
                    if noscores:
                        return pt4[t % 4]
                    pt = ptp.tile([128, 2, 512], FP8, tag="pt", name="pt")
                    hh = h % 2
                    for u in range(2):
                        kb = 2 * t + u
                        st = ps_st.tile([128, 512], F32, tag="st")
                        if SC_DR:
                            nc.tensor.matmul(
                                st,
                                lhsT=k_all[64 * hh:64 * (hh + 1), h // 2, :,
                                           128 * kb:128 * (kb + 1)],
                                rhs=q_bcast(h, ib),
                                start=True, stop=True, perf_mode=DR)
                        else:
                            nc.tensor.matmul(
                                st,
                                lhsT=k_all[64 * hh:64 * (hh + 1), h // 2, 0,
                                           128 * kb:128 * (kb + 1)],
                                rhs=q_pair[h // 2][
                                    64 * hh:64 * (hh + 1),
                                    512 * ib:512 * (ib + 1)],
                                start=True, stop=True)
                        if EXP_PAT[kb] == "A":
                            nc.scalar.activation(out=pt[:, u], in_=st,
                                                 func=AF.Exp,
                                                 scale=ATT_SCALE)
                        else:
                            nc.vector.tensor_scalar(
                                out=pt[:, u].bitcast(U8), in0=st,
                                scalar1=EC1, scalar2=EC2,
                                op0=OP.mult, op1=OP.add)
                    return pt

                def av_step(t, pt, av=av, h=h):
                    if noav:
                        return
                    if densepe:
                        pt = pt4[t % 4]
                    for r in range(AV_REPEAT):
                        nc.tensor.matmul(
                            av, lhsT=vt_sb[:, 2 * t:2 * t + 2, h, 0:65],
                            rhs=pt,
                            start=(t == 0 and r == 0),
                            stop=(t == 15 and r == AV_REPEAT - 1),
                            perf_mode=DR)

                for t in range(16):
                    pend.append((t, sc_pair(t)))
                    if t == 4 and pend_norm is not None:
                        pend_norm()
                        pend_norm = None
                    if t == 13 and pend_proj is not None:
                        pend_proj()
                        pend_proj = None
                    if t >= AV_DEFER and (t - AV_DEFER) % AV_GROUP == \
                            AV_GROUP - 1:
                        for _ in range(AV_GROUP):
                            av_step(*pend.pop(0))
                while pend:
                    av_step(*pend.pop(0))
                if noav:
                    continue

                # normalize: at2[d, i] = av[d, i] * (32 / den[i]).  The
                # reciprocal + broadcast issue now; the multiply is deferred
                # into the next head's window to hide the broadcast latency.
                rec_b = work.tile([65, 512], BF16, tag="rec_b")
                with nc.allow_low_precision(reason="bf16 softmax denom"):
                    nc.vector.reciprocal(out=rec_b[64:65], in_=av[64:65])
                last = ib == 3 and h == 3
                if NORM_MODE == "pe" or last:
                    # broadcast on the PE (outer product into a spare av
                    # bank) and stage through SBUF for the multiply: no DMA
                    # and no 900ns DMA-completion semaphore in the window
                    # steady state.
                    rbq = ps_av.tile([65, 512], F32, tag="av", name="rbq")
                    nc.tensor.matmul(rbq[0:64], lhsT=ones64b[64:65, :],
                                     rhs=rec_b[64:65], start=True, stop=True)

                    def mult(av=av, rbq=rbq, h=h, at2=at2):
                        rbs = work.tile([64, 512], BF16, tag="rbs")
                        nc.vector.tensor_copy(out=rbs, in_=rbq[0:64])
                        nc.vector.tensor_mul(
                            out=at2[64 * (h % 2):64 * (h % 2) + 64, h // 2],
                            in0=av[0:64], in1=rbs)
                    if last:
                        mult()
                        break
                else:
                    rb = rbp.tile([64, 512], BF16, tag="rb")
                    rsrc = rec_b[64:65]
                    nc.sync.dma_start(out=rb, in_=bass.AP(
                        tensor=rsrc.tensor, offset=rsrc.offset,
                        ap=[list(rsrc.ap[0]), [0, 64]] + list(rsrc.ap[1:])))

                    def mult(av=av, rb=rb, h=h, at2=at2):
                        nc.vector.tensor_mul(
                            out=at2[64 * (h % 2):64 * (h % 2) + 64, h // 2],
                            in0=av[0:64], in1=rb)
                pend_norm = mult

            if not noav:
                pend_proj = make_proj(ib, at2)
        if pend_norm is not None:
            pend_norm()
            pend_norm = None
        if pend_proj is not None:
            pend_proj()


def build_program(split_waits=True, iters=1, mode="full"):
    _apply_drain_patch()
    nc = bass.Bass()
    xbf = nc.declare_dram_parameter("x_bf16", [C, N], BF16, isOutput=False)
    xf8 = nc.declare_dram_parameter("x_fp8", [C, N], FP8, isOutput=False)
    wqkv = nc.declare_dram_parameter("w_qkvT", [C, 3 * C], F32, isOutput=False)
    wproj = nc.declare_dram_parameter("w_projT", [C, C], F32, isOutput=False)
    gam = nc.declare_dram_parameter("gn_gamma", [C], F32, isOutput=False)
    bet = nc.declare_dram_parameter("gn_beta", [C], F32, isOutput=False)
    bpr = nc.declare_dram_parameter("b_proj", [C], F32, isOutput=False)
    inda = nc.declare_dram_parameter("indA", [C, 8], F32, isOutput=False)
    indb = nc.declare_dram_parameter("indB", [8, C], F32, isOutput=False)
    zr8 = nc.declare_dram_parameter("zeros8", [128, N], mybir.dt.uint8,
                                    isOutput=False)
    out = nc.declare_dram_parameter("out", [C, NQ], F32, isOutput=True)
    with tile.TileContext(nc) as tc:
        for _ in range(iters):
            _body(tc, xbf, xf8, wqkv, wproj, gam, bet, bpr, inda, indb, zr8,
                  out, mode=mode)
    if split_waits:
        _split_excess_waits(nc)
    return nc


def make_in_maps(x, gn_gamma, gn_beta, w_qkv, w_proj, b_proj):
    x = np.ascontiguousarray(x, dtype=np.float32)
    w_qkvT = np.ascontiguousarray(np.asarray(w_qkv, np.float32).T)
    w_projT = np.ascontiguousarray(np.asarray(w_proj, np.float32).T)
    gn_gamma = np.ascontiguousarray(gn_gamma, dtype=np.float32)
    gn_beta = np.ascontiguousarray(gn_beta, dtype=np.float32)
    b_proj = np.ascontiguousarray(b_proj, dtype=np.float32)
    ch_groups = np.arange(C) // 32
    indA = np.zeros((C, 8), np.float32)
    indA[np.arange(C), ch_groups] = 1.0 / 32.0
    indB = np.zeros((8, C), np.float32)
    indB[ch_groups, np.arange(C)] = 1.0
    in_maps = []
    bf16_np = mybir.dt.np(BF16)
    fp8_np = mybir.dt.np(FP8)
    for core in range(NCORES):
        s, half = core // 2, core % 2
        xfl = x[s].reshape(C, N)
        x_core = np.ascontiguousarray(np.concatenate(
            [xfl[:, half * NQ:(half + 1) * NQ],
             xfl[:, (1 - half) * NQ:(2 - half) * NQ]], axis=1))
        in_maps.append({
            "x_bf16": x_core.astype(bf16_np),
            "x_fp8": x_core.astype(fp8_np),
            "w_qkvT": w_qkvT,
            "w_projT": w_projT,
            "gn_gamma": gn_gamma,
            "gn_beta": gn_beta,
            "b_proj": b_proj,
            "indA": indA,
            "indB": indB,
            "zeros8": np.zeros((128, N), np.uint8),
        })
    return in_maps


def assemble_output(results):
    out = np.empty((B, C, N), np.float32)
    for core in range(NCORES):
        s, half = core // 2, core % 2
        out[s][:, half * NQ:(half + 1) * NQ] = results[core]["out"]
    return out.reshape(B, C, HGT, WID)


_PROGRAM_CACHE = {}


def kernel(x, gn_gamma, gn_beta, w_qkv, w_proj, b_proj):
    if "nc" not in _PROGRAM_CACHE:
        _PROGRAM_CACHE["nc"] = build_program()
    nc = _PROGRAM_CACHE["nc"]
    in_maps = make_in_maps(x, gn_gamma, gn_beta, w_qkv, w_proj, b_proj)
    res = run_bass_kernel_spmd(nc, in_maps, list(range(NCORES)))
    return assemble_output(res.results)

